# revision 1
# baseline (speedup 1.0000x reference)
"""Trainium2 Bass kernel for nn_MeshUpConv (MeshCNN up-conv block).

Strategy: data-parallel over batch B=8 (one mesh per NeuronCore).
Per core, each mesh_conv is computed as:
  - neighbor features fetched with SWDGE dma_gather(transpose=True) from a
    row-major bf16 table in DRAM (gather lands channel-major, matmul-ready)
  - symmetric features built on DVE (add/sub/abs)
  - 5*C-wide matmul on PE in bf16, accumulated in PSUM f32
  - instance-norm stats accumulated via ACT accum_out during the PSUM->SBUF
    copy; normalize+relu(+residual) in a second pass
  - row-major gather tables for the next conv produced via PE transposes
"""

import sys

for _p in ("/opt/trn_rl_repo",):
    if _p not in sys.path:
        sys.path.append(_p)

import numpy as np
import ml_dtypes

BF16 = ml_dtypes.bfloat16

B = 8
E_FULL = 16384
CIN = 128
CO = 256
OB = 2          # output channel blocks of 128
EC = 512        # edges per chunk
EPS = 1e-5
USE_SWDGE_CAST = True   # False: stage f32 via sync DMA + ACT cast instead
GNI = 512               # idxs per dma_gather (ring-size limited)


def _pack_idx(ei: np.ndarray, E: int) -> np.ndarray:
    """ei [E,4] int32 -> [128, NCH*128] int16 wrapped gather-index layout.

    Per chunk c the 2048 indices are ordered j = s*EC + i (slot-major), and
    index j lives at [16*g + j%16, c*128 + j//16] for every g in 0..7.
    """
    nch = E // EC
    arr = ei.reshape(nch, EC, 4).transpose(0, 2, 1).reshape(nch, 4 * EC)
    w = arr.reshape(nch, (4 * EC) // 16, 16).transpose(2, 0, 1).reshape(16, -1)
    return np.tile(w, (8, 1)).astype(np.int16)


def _pack_w(W: np.ndarray) -> np.ndarray:
    """W [256, C, 5] f32 -> [128, NBLK*128] bf16 lhsT blocks ordered (ob,k,cb)."""
    O, C, K = W.shape
    cb_n = C // 128
    out = np.empty((128, OB * K * cb_n * 128), np.float32)
    n = 0
    for ob in range(OB):
        for k in range(K):
            for cb in range(cb_n):
                blk = W[ob * 128:(ob + 1) * 128, cb * 128:(cb + 1) * 128, k].T
                out[:, n * 128:(n + 1) * 128] = blk
                n += 1
    return out.astype(BF16)


def _pack_b(b: np.ndarray) -> np.ndarray:
    return np.asarray(b).reshape(OB, 128).T.astype(np.float32).copy()


def build_nc(E: int = E_FULL):
    import concourse.bacc as bacc
    import concourse.mybir as mybir
    from concourse.tile import TileContext
    from concourse.tile_rust import add_dep_helper

    dt = mybir.dt
    Alu = mybir.AluOpType
    Act = mybir.ActivationFunctionType
    NCH = E // EC

    nc = bacc.Bacc("TRN2")

    fu = nc.dram_tensor("from_up", [CIN, E], dt.float32, kind="ExternalInput")
    fd = nc.dram_tensor("from_down", [CO, E], dt.float32, kind="ExternalInput")
    idx = nc.dram_tensor("idx", [128, NCH * 128], dt.int16, kind="ExternalInput")
    wup = nc.dram_tensor("wup", [128, 10 * 128], dt.bfloat16, kind="ExternalInput")
    w1 = nc.dram_tensor("w1", [128, 40 * 128], dt.bfloat16, kind="ExternalInput")
    w2a = nc.dram_tensor("w2a", [128, 20 * 128], dt.bfloat16, kind="ExternalInput")
    w2b = nc.dram_tensor("w2b", [128, 20 * 128], dt.bfloat16, kind="ExternalInput")
    bia = nc.dram_tensor("bia", [128, 4 * OB], dt.float32, kind="ExternalInput")
    ident = nc.dram_tensor("ident", [128, 128], dt.bfloat16, kind="ExternalInput")
    out = nc.dram_tensor("out", [CO, E], dt.float32, kind="ExternalOutput")

    fu_rm = nc.dram_tensor("fu_rm", [E, CIN], dt.bfloat16, kind="Internal")
    rm1 = nc.dram_tensor("rm1", [E, 2 * CO], dt.bfloat16, kind="Internal")
    rm2 = nc.dram_tensor("rm2", [E, CO], dt.bfloat16, kind="Internal")
    rm3 = nc.dram_tensor("rm3", [E, CO], dt.bfloat16, kind="Internal")

    # The xbar transpose-mode of dma_gather(transpose=True) deadlocks the
    # SDMA engines when a plain DMA copy runs concurrently (the known
    # DMATranspose/DMACopy HW hazard; Tile serializes it for dma_start
    # transposes but not for InstDMAGatherAnt). Fence: every gather depends
    # on all DMAs issued since the previous gather, and every later DMA
    # depends on the most recent gather.
    _pending = []
    _last_gather = [None]

    def _dma(inst):
        if _last_gather[0] is not None:
            add_dep_helper(inst.ins, _last_gather[0].ins,
                           reason="dma-after-gather-fence")
        _pending.append(inst)
        return inst

    def _gather(inst):
        for d in _pending:
            add_dep_helper(inst.ins, d.ins, reason="gather-fence")
        _pending.clear()
        if _last_gather[0] is not None:
            add_dep_helper(inst.ins, _last_gather[0].ins, reason="gather-chain")
        _last_gather[0] = inst
        return inst

    with TileContext(nc) as tc:
        with (
            tc.tile_pool(name="persist", bufs=1) as persist,
            tc.tile_pool(name="wp", bufs=1) as wpool,
            tc.tile_pool(name="gp", bufs=2) as gpool,
            tc.tile_pool(name="dp", bufs=2) as dpool,
            tc.tile_pool(name="cp", bufs=2) as cpool,
            tc.tile_pool(name="rp", bufs=2) as rpool,
            tc.tile_pool(name="bp", bufs=2) as bpool,
            tc.tile_pool(name="jkp", bufs=1) as jkpool,
            tc.tile_pool(name="ixp", bufs=2) as ixpool,
            tc.tile_pool(name="mmps", bufs=4, space="PSUM") as mmps,
            tc.tile_pool(name="tpps", bufs=4, space="PSUM") as tpps,
        ):
            bufA = persist.tile([128, OB * E], dt.bfloat16, tag="bufA")
            bufB = persist.tile([128, OB * E], dt.bfloat16, tag="bufB")
            id_t = persist.tile([128, 128], dt.bfloat16, tag="ident")
            bias_t = persist.tile([128, 4 * OB], dt.float32, tag="bias")
            ssum = persist.tile([128, OB * NCH], dt.float32, tag="ssum")
            ssq = persist.tile([128, OB * NCH], dt.float32, tag="ssq")
            nrm = persist.tile([128, 8 * OB], dt.float32, tag="nrm")

            _dma(nc.sync.dma_start(id_t[:], ident[:]))
            _dma(nc.sync.dma_start(bias_t[:], bia[:]))

            def cast_load(dst_bf16_ap, src_f32_ap, n):
                """DRAM f32 -> SBUF bf16 [128, n]."""
                if USE_SWDGE_CAST:
                    _dma(nc.gpsimd.dma_start(dst_bf16_ap, src_f32_ap))
                else:
                    cf = cpool.tile([128, EC], dt.float32, tag="cf")
                    _dma(nc.sync.dma_start(cf[:, 0:n], src_f32_ap))
                    nc.scalar.activation(dst_bf16_ap, cf[:, 0:n], Act.Copy)

            def transpose_to(rt, src_ap, ob):
                """PE-transpose src [128ch,128e] -> rt[:, ob*128 : +128]."""
                tp = tpps.tile([128, 128], dt.bfloat16, tag="tp")
                nc.tensor.transpose(tp[:], src_ap, id_t[:])
                nc.vector.tensor_copy(rt[:, ob * 128:(ob + 1) * 128], tp[:])

            # ---------------- P0a: fu_rm = from_up.T (bf16) ----------------
            for c in range(NCH):
                e0 = c * EC
                ct = cpool.tile([128, EC], dt.bfloat16, tag="ct")
                cast_load(ct[:], fu[:, e0:e0 + EC], EC)
                for g in range(EC // 128):
                    eg = e0 + g * 128
                    rt = rpool.tile([128, CIN], dt.bfloat16, tag="rt_u")
                    transpose_to(rt, ct[:, g * 128:(g + 1) * 128], 0)
                    _dma(nc.scalar.dma_start(fu_rm[eg:eg + 128, :], rt[:]))

            # ------------- P0b: rm1[:, 256:512] = from_down.T --------------
            for c in range(NCH):
                e0 = c * EC
                cts = []
                for ob in range(OB):
                    ct = cpool.tile([128, EC], dt.bfloat16, tag="ct")
                    cast_load(ct[:], fd[ob * 128:(ob + 1) * 128, e0:e0 + EC], EC)
                    cts.append(ct)
                for g in range(EC // 128):
                    eg = e0 + g * 128
                    rt = rpool.tile([128, CO], dt.bfloat16, tag="rt")
                    for ob in range(OB):
                        transpose_to(rt, cts[ob][:, g * 128:(g + 1) * 128], ob)
                    _dma(nc.scalar.dma_start(rm1[eg:eg + 128, CO:2 * CO], rt[:]))

            # ------------------------ conv pass A ---------------------------
            def conv_pass_a(CB, table, table_c, w_dram, bias_col, center_fn,
                            raw_dst, stats):
                nblk = OB * 5 * CB
                w_t = wpool.tile([128, nblk * 128], dt.bfloat16, tag="w")
                _dma(nc.sync.dma_start(w_t[:], w_dram[:]))
                nsplit = (4 * EC) // GNI
                for c in range(NCH):
                    e0 = c * EC
                    ix = ixpool.tile([128, 128], dt.int16, tag="ix")
                    _dma(nc.sync.dma_start(ix[:], idx[:, c * 128:(c + 1) * 128]))
                    gts = []
                    for g in range(nsplit):
                        gt = gpool.tile([128, CB * GNI], dt.bfloat16,
                                        tag=f"gd{g}")
                        gt3 = gt[:].rearrange("p (f n) -> p f n", f=CB)
                        _gather(nc.gpsimd.dma_gather(
                            gt3, table[:],
                            ix[:, g * (GNI // 16):(g + 1) * (GNI // 16)],
                            num_idxs=GNI, num_idxs_reg=GNI,
                            elem_size=table_c, transpose=True,
                        ))
                        gts.append(gt3)

                    def slot(s):
                        k = (s * EC) // GNI
                        off = (s * EC) % GNI
                        return gts[k][:, :, off:off + EC]
                    ct = center_fn(c)
                    dt1 = dpool.tile([128, CB * EC], dt.bfloat16, tag="dt1")
                    dt2 = dpool.tile([128, CB * EC], dt.bfloat16, tag="dt2")
                    s = slot
                    d1v = dt1[:].rearrange("p (f n) -> p f n", f=CB)
                    d2v = dt2[:].rearrange("p (f n) -> p f n", f=CB)
                    nc.vector.tensor_tensor(d1v, s(0), s(2), op=Alu.subtract)
                    nc.vector.tensor_tensor(d2v, s(1), s(3), op=Alu.subtract)
                    nc.vector.tensor_tensor(s(0), s(0), s(2), op=Alu.add)
                    nc.vector.tensor_tensor(s(1), s(1), s(3), op=Alu.add)
                    d1i = dt1[:].bitcast(dt.int16)
                    d2i = dt2[:].bitcast(dt.int16)
                    nc.vector.tensor_scalar(d1i, d1i, 0x7FFF, None,
                                            op0=Alu.bitwise_and)
                    nc.vector.tensor_scalar(d2i, d2i, 0x7FFF, None,
                                            op0=Alu.bitwise_and)

                    for ob in range(OB):
                        ps = mmps.tile([128, EC], dt.float32, tag="ps")
                        nmm = 5 * CB
                        i_mm = 0
                        for k in range(5):
                            for cb in range(CB):
                                if k == 0:
                                    rhs = ct(cb)
                                elif k == 1:
                                    rhs = slot(0)[:, cb, :]
                                elif k == 2:
                                    rhs = slot(1)[:, cb, :]
                                elif k == 3:
                                    rhs = dt1[:, cb * EC:(cb + 1) * EC]
                                else:
                                    rhs = dt2[:, cb * EC:(cb + 1) * EC]
                                n = (ob * 5 + k) * CB + cb
                                nc.tensor.matmul(
                                    ps[:], w_t[:, n * 128:(n + 1) * 128], rhs,
                                    start=(i_mm == 0), stop=(i_mm == nmm - 1),
                                )
                                i_mm += 1
                        bias_ap = bias_t[:, bias_col * OB + ob:
                                         bias_col * OB + ob + 1]
                        raw_ap = raw_dst[:, ob * E + e0:ob * E + e0 + EC]
                        if stats:
                            nc.scalar.activation(
                                raw_ap, ps[:], Act.Identity, bias=bias_ap,
                                accum_out=ssum[:, ob * NCH + c:ob * NCH + c + 1],
                            )
                            jk = jkpool.tile([128, EC], dt.bfloat16, tag="jk")
                            nc.vector.scalar_tensor_tensor(
                                jk[:], raw_ap, 1.0, raw_ap,
                                op0=Alu.mult, op1=Alu.mult,
                                accum_out=ssq[:, ob * NCH + c:ob * NCH + c + 1],
                            )
                        else:
                            nc.scalar.activation(
                                raw_ap, ps[:], Act.Identity, bias=bias_ap,
                            )

            # -------------------- stats finalize ---------------------------
            def conv_finalize(slot):
                mean = nrm[:, 0:OB]
                var = nrm[:, OB:2 * OB]
                scal = nrm[:, (2 + 2 * slot) * OB:(3 + 2 * slot) * OB]
                shift = nrm[:, (3 + 2 * slot) * OB:(4 + 2 * slot) * OB]
                for ob in range(OB):
                    nc.vector.reduce_sum(
                        mean[:, ob:ob + 1], ssum[:, ob * NCH:(ob + 1) * NCH],
                        axis=mybir.AxisListType.X)
                    nc.vector.reduce_sum(
                        var[:, ob:ob + 1], ssq[:, ob * NCH:(ob + 1) * NCH],
                        axis=mybir.AxisListType.X)
                nc.vector.tensor_scalar(mean, mean, 1.0 / E, None, op0=Alu.mult)
                nc.vector.tensor_scalar(var, var, 1.0 / E, None, op0=Alu.mult)
                # var <- var - mean^2; then scal = 1/sqrt(var+eps)
                nc.vector.scalar_tensor_tensor(
                    shift, mean, -1.0, mean, op0=Alu.mult, op1=Alu.mult)
                nc.vector.tensor_tensor(var, var, shift, op=Alu.add)
                nc.vector.tensor_scalar(var, var, EPS, None, op0=Alu.add)
                nc.scalar.activation(var, var, Act.Sqrt)
                nc.vector.reciprocal(scal, var)
                nc.vector.scalar_tensor_tensor(
                    shift, mean, -1.0, scal, op0=Alu.mult, op1=Alu.mult)
                return scal, shift

            # ------------------------- up conv ------------------------------
            def up_center(c):
                ct = cpool.tile([128, EC], dt.bfloat16, tag="ct")
                cast_load(ct[:], fu[:, c * EC:c * EC + EC], EC)
                return lambda cb: ct[:]

            conv_pass_a(1, fu_rm, CIN, wup, 0, up_center, bufB, stats=False)

            # x1up -> rm1[:, 0:256] (transpose from bufB)
            for c in range(NCH):
                e0 = c * EC
                for g in range(EC // 128):
                    eg = e0 + g * 128
                    rt = rpool.tile([128, CO], dt.bfloat16, tag="rt")
                    for ob in range(OB):
                        transpose_to(
                            rt, bufB[:, ob * E + eg:ob * E + eg + 128], ob)
                    _dma(nc.scalar.dma_start(rm1[eg:eg + 128, 0:CO], rt[:]))

            # ------------------------- conv1 --------------------------------
            def c1_center(c):
                e0 = c * EC
                ctf = cpool.tile([128, 2 * EC], dt.bfloat16, tag="ct")
                for ob in range(OB):
                    cast_load(ctf[:, ob * EC:(ob + 1) * EC],
                              fd[ob * 128:(ob + 1) * 128, e0:e0 + EC], EC)

                def get(cb):
                    if cb < 2:
                        return bufB[:, cb * E + e0:cb * E + e0 + EC]
                    return ctf[:, (cb - 2) * EC:(cb - 1) * EC]
                return get

            conv_pass_a(4, rm1, 2 * CO, w1, 1, c1_center, bufA, stats=True)

            # c1B: x1n = relu(norm(raw1)) -> bufB ; transposes -> rm2
            scal, shift = conv_finalize(0)
            for c in range(NCH):
                e0 = c * EC
                for ob in range(OB):
                    nc.scalar.activation(
                        bufB[:, ob * E + e0:ob * E + e0 + EC],
                        bufA[:, ob * E + e0:ob * E + e0 + EC],
                        Act.Relu, bias=shift[:, ob:ob + 1],
                        scale=scal[:, ob:ob + 1])
                for g in range(EC // 128):
                    eg = e0 + g * 128
                    rt = rpool.tile([128, CO], dt.bfloat16, tag="rt")
                    for ob in range(OB):
                        transpose_to(
                            rt, bufB[:, ob * E + eg:ob * E + eg + 128], ob)
                    _dma(nc.scalar.dma_start(rm2[eg:eg + 128, :], rt[:]))

            # ------------------------- conv2a -------------------------------
            def c2a_center(c):
                e0 = c * EC
                return lambda cb: bufB[:, cb * E + e0:cb * E + e0 + EC]

            conv_pass_a(2, rm2, CO, w2a, 2, c2a_center, bufA, stats=True)

            # c2aB: x2 = relu(norm(raw2a) + x1n) -> bufB ; transposes -> rm3
            scal, shift = conv_finalize(1)
            for c in range(NCH):
                e0 = c * EC
                for ob in range(OB):
                    t = bpool.tile([128, EC], dt.bfloat16, tag="bt")
                    nc.scalar.activation(
                        t[:], bufA[:, ob * E + e0:ob * E + e0 + EC],
                        Act.Identity, bias=shift[:, ob:ob + 1],
                        scale=scal[:, ob:ob + 1])
                    nc.vector.tensor_tensor(
                        t[:], t[:], bufB[:, ob * E + e0:ob * E + e0 + EC],
                        op=Alu.add)
                    nc.vector.tensor_scalar(
                        bufB[:, ob * E + e0:ob * E + e0 + EC], t[:], 0.0, None,
                        op0=Alu.max)
                for g in range(EC // 128):
                    eg = e0 + g * 128
                    rt = rpool.tile([128, CO], dt.bfloat16, tag="rt")
                    for ob in range(OB):
                        transpose_to(
                            rt, bufB[:, ob * E + eg:ob * E + eg + 128], ob)
                    _dma(nc.scalar.dma_start(rm3[eg:eg + 128, :], rt[:]))

            # ------------------------- conv2b -------------------------------
            conv_pass_a(2, rm3, CO, w2b, 3, c2a_center, bufA, stats=True)

            # c2bB: out = relu(norm(raw2b) + x2) -> DRAM f32
            scal, shift = conv_finalize(2)
            for c in range(NCH):
                e0 = c * EC
                for ob in range(OB):
                    t = bpool.tile([128, EC], dt.bfloat16, tag="bt")
                    nc.scalar.activation(
                        t[:], bufA[:, ob * E + e0:ob * E + e0 + EC],
                        Act.Identity, bias=shift[:, ob:ob + 1],
                        scale=scal[:, ob:ob + 1])
                    u = bpool.tile([128, EC], dt.float32, tag="ut")
                    nc.vector.tensor_tensor(
                        u[:], t[:], bufB[:, ob * E + e0:ob * E + e0 + EC],
                        op=Alu.add)
                    nc.vector.tensor_scalar(u[:], u[:], 0.0, None, op0=Alu.max)
                    _dma(nc.scalar.dma_start(
                        out[ob * 128:(ob + 1) * 128, e0:e0 + EC], u[:]))

    nc.finalize()
    return nc


_NC_CACHE = {}


def _get_nc(E):
    if E not in _NC_CACHE:
        _NC_CACHE[E] = build_nc(E)
    return _NC_CACHE[E]


def make_in_maps(from_up, from_down, edge_index, W_up, b_up, W1, b1, W2a, b2a,
                 W2b, b2b, E=E_FULL):
    """Build the per-core input maps (host-side sharding + layout packing)."""
    wup_p = _pack_w(np.asarray(W_up))
    w1_p = _pack_w(np.asarray(W1))
    w2a_p = _pack_w(np.asarray(W2a))
    w2b_p = _pack_w(np.asarray(W2b))
    bia_p = np.concatenate(
        [_pack_b(b_up), _pack_b(b1), _pack_b(b2a), _pack_b(b2b)], axis=1)
    ident = np.eye(128, dtype=BF16)
    in_maps = []
    for i in range(B):
        in_maps.append({
            "from_up": np.ascontiguousarray(from_up[i], np.float32),
            "from_down": np.ascontiguousarray(from_down[i], np.float32),
            "idx": _pack_idx(np.asarray(edge_index[i]), E),
            "wup": wup_p, "w1": w1_p, "w2a": w2a_p, "w2b": w2b_p,
            "bia": bia_p, "ident": ident,
        })
    return in_maps


def kernel(from_up, from_down, edge_index, W_up, b_up, W1, b1, W2a, b2a,
           W2b, b2b) -> np.ndarray:
    from concourse import bass_utils

    nc = _get_nc(E_FULL)
    in_maps = make_in_maps(from_up, from_down, edge_index, W_up, b_up,
                           W1, b1, W2a, b2a, W2b, b2b)
    res = bass_utils.run_bass_kernel_spmd(nc, in_maps, core_ids=list(range(B)))
    return np.stack([r["out"] for r in res.results]).astype(np.float32)



# revision 7
# speedup vs baseline: 1.0685x; 1.0685x over previous
"""Trainium2 Bass kernel for nn_MeshUpConv (MeshCNN up-conv block).

Strategy: data-parallel over batch B=8 (one mesh per NeuronCore).

v2 changes vs baseline:
  - up-conv neighbor features are host-gathered (pure input rearrangement)
    and shipped channel-major in bf16 -> no on-device gathers for up conv,
    no fu_rm table, no input-transpose passes.
  - from_down shipped pre-cast bf16 both channel-major (centers) and
    row-major (rm1 fd-half, copied DRAM->DRAM via SBUF bounce).
  - remaining dma_gathers (conv1/conv2a/conv2b) spread across 4 SWDGE
    queues -> descriptor generation runs on 4 Q7 core pairs in parallel
    (it was the serialized bottleneck: ~9ns/row on one pair).
  - all idx chunks prefetched once (idx identical across convs).
  - feature build writes sums/absdiffs in place into the gather tiles
    (d = a-b; a' = 2a-d; |d|), no separate diff tiles.
  - plain DMAs on sync engine; gather<->plain fence kept (xbar hazard),
    per-queue gather chaining only.
"""

import sys

for _p in ("/opt/trn_rl_repo",):
    if _p not in sys.path:
        sys.path.append(_p)

import numpy as np
import ml_dtypes

BF16 = ml_dtypes.bfloat16

B = 8
E_FULL = 16384
CIN = 128
CO = 256
OB = 2          # output channel blocks of 128
EC = 512        # edges per chunk
EPS = 1e-5
GNI = 512       # idxs per dma_gather (ring-size limited)
NQ = 4          # SWDGE queues
SAFE_CHAIN = True   # chain all gathers globally (baseline hazard discipline)


def _pack_idx(ei: np.ndarray, E: int) -> np.ndarray:
    """ei [E,4] int32 -> [128, NCH*128] int16 wrapped gather-index layout.

    Per chunk c the 2048 indices are ordered j = s*EC + i (slot-major), and
    index j lives at [16*g + j%16, c*128 + j//16] for every g in 0..7.
    """
    nch = E // EC
    arr = ei.reshape(nch, EC, 4).transpose(0, 2, 1).reshape(nch, 4 * EC)
    w = arr.reshape(nch, (4 * EC) // 16, 16).transpose(2, 0, 1).reshape(16, -1)
    return np.tile(w, (8, 1)).astype(np.int16)


def _pack_w(W: np.ndarray) -> np.ndarray:
    """W [256, C, 5] f32 -> [128, NBLK*128] bf16 lhsT blocks ordered (ob,k,cb)."""
    O, C, K = W.shape
    cb_n = C // 128
    out = np.empty((128, OB * K * cb_n * 128), np.float32)
    n = 0
    for ob in range(OB):
        for k in range(K):
            for cb in range(cb_n):
                blk = W[ob * 128:(ob + 1) * 128, cb * 128:(cb + 1) * 128, k].T
                out[:, n * 128:(n + 1) * 128] = blk
                n += 1
    return out.astype(BF16)


def _pack_b(b: np.ndarray) -> np.ndarray:
    return np.asarray(b).reshape(OB, 128).T.astype(np.float32).copy()


def build_nc(E: int = E_FULL):
    import concourse.bacc as bacc
    import concourse.mybir as mybir
    from concourse.tile import TileContext
    from concourse.tile_rust import add_dep_helper

    dt = mybir.dt
    Alu = mybir.AluOpType
    Act = mybir.ActivationFunctionType
    NCH = E // EC

    nc = bacc.Bacc("TRN2", num_swdge_queues=NQ)

    nbup = nc.dram_tensor("nbup", [128, 4, E], dt.bfloat16, kind="ExternalInput")
    fu = nc.dram_tensor("fu", [128, E], dt.bfloat16, kind="ExternalInput")
    fd = nc.dram_tensor("fd", [128, 2, E], dt.bfloat16, kind="ExternalInput")
    fdrm = nc.dram_tensor("fdrm", [E, CO], dt.bfloat16, kind="ExternalInput")
    idx = nc.dram_tensor("idx", [128, NCH * 128], dt.int16, kind="ExternalInput")
    wup = nc.dram_tensor("wup", [128, 10 * 128], dt.bfloat16, kind="ExternalInput")
    w1 = nc.dram_tensor("w1", [128, 40 * 128], dt.bfloat16, kind="ExternalInput")
    w2a = nc.dram_tensor("w2a", [128, 20 * 128], dt.bfloat16, kind="ExternalInput")
    w2b = nc.dram_tensor("w2b", [128, 20 * 128], dt.bfloat16, kind="ExternalInput")
    bia = nc.dram_tensor("bia", [128, 4 * OB], dt.float32, kind="ExternalInput")
    ident = nc.dram_tensor("ident", [128, 128], dt.bfloat16, kind="ExternalInput")
    out = nc.dram_tensor("out", [CO, E], dt.float32, kind="ExternalOutput")

    rm1 = nc.dram_tensor("rm1", [E, 2 * CO], dt.bfloat16, kind="Internal")
    rm2 = nc.dram_tensor("rm2", [E, CO], dt.bfloat16, kind="Internal")
    rm3 = nc.dram_tensor("rm3", [E, CO], dt.bfloat16, kind="Internal")

    # xbar-transpose gathers deadlock the SDMA engines when a plain DMA copy
    # runs concurrently. Fence: every gather group depends on all plain DMAs
    # issued since the previous group, and every later plain DMA depends on
    # the most recent gather of each queue. Same-queue gathers are ordered by
    # the per-queue ring FIFO; the 4 queues' descriptor generation runs on 4
    # Q7 core pairs in parallel.
    _pending = []
    _last_gather = [None] * NQ

    def _dma(inst):
        for q in range(NQ):
            if _last_gather[q] is not None:
                add_dep_helper(inst.ins, _last_gather[q].ins,
                               reason="dma-after-gather-fence")
        _pending.append(inst)
        return inst

    def _gather_group(insts):
        for inst in insts:
            for d in _pending:
                add_dep_helper(inst.ins, d.ins, reason="gather-fence")
        _pending.clear()
        if SAFE_CHAIN:
            prev = _last_gather[0]
            for inst in insts:
                if prev is not None:
                    add_dep_helper(inst.ins, prev.ins, reason="gather-chain")
                prev = inst
            _last_gather[0] = prev
        else:
            for q, inst in enumerate(insts):
                _last_gather[q] = inst

    with TileContext(nc) as tc:
        with (
            tc.tile_pool(name="persist", bufs=1) as persist,
            tc.tile_pool(name="wp", bufs=1) as wpool,
            tc.tile_pool(name="gp", bufs=2) as gpool,
            tc.tile_pool(name="cp", bufs=2) as cpool,
            tc.tile_pool(name="rp", bufs=2) as rpool,
            tc.tile_pool(name="bp", bufs=2) as bpool,
            tc.tile_pool(name="jkp", bufs=1) as jkpool,
            tc.tile_pool(name="mmps", bufs=4, space="PSUM") as mmps,
            tc.tile_pool(name="tpps", bufs=2, space="PSUM") as tpps,
        ):
            bufA = persist.tile([128, OB * E], dt.bfloat16, tag="bufA")
            bufB = persist.tile([128, OB * E], dt.bfloat16, tag="bufB")
            id_t = persist.tile([128, 128], dt.bfloat16, tag="ident")
            bias_t = persist.tile([128, 4 * OB], dt.float32, tag="bias")
            ssum = persist.tile([128, OB * NCH], dt.float32, tag="ssum")
            ssq = persist.tile([128, OB * NCH], dt.float32, tag="ssq")
            nrm = persist.tile([128, 8 * OB], dt.float32, tag="nrm")
            ix_all = persist.tile([128, NCH * 128], dt.int16, tag="ix")

            _dma(nc.sync.dma_start(id_t[:], ident[:]))
            _dma(nc.sync.dma_start(bias_t[:], bia[:]))
            _dma(nc.sync.dma_start(ix_all[:], idx[:]))

            # ---------------- shared epilogue: transposes -> rm -------------
            def transpose_rows(src_buf, e0, rm_dst, col0, ncol_ob):
                """PE-transpose src_buf chunk (both ob) -> rm rows e0..e0+EC."""
                tps = []
                for ob in range(ncol_ob):
                    tp = tpps.tile([128, EC], dt.bfloat16, tag=f"tp{ob}")
                    for g in range(EC // 128):
                        nc.tensor.transpose(
                            tp[:, g * 128:(g + 1) * 128],
                            src_buf[:, ob * E + e0 + g * 128:
                                    ob * E + e0 + g * 128 + 128],
                            id_t[:])
                    tps.append(tp)
                rtc = rpool.tile([128, EC // 128, ncol_ob * 128], dt.bfloat16,
                                 tag="rtc")
                for ob in range(ncol_ob):
                    nc.vector.tensor_copy(
                        rtc[:].rearrange("p g (o n) -> p g o n", o=ncol_ob)
                        [:, :, ob, :],
                        tps[ob][:].rearrange("p (g n) -> p g n", g=EC // 128))
                for g in range(EC // 128):
                    eg = e0 + g * 128
                    _dma(nc.sync.dma_start(
                        rm_dst[eg:eg + 128, col0:col0 + ncol_ob * 128],
                        rtc[:, g, :]))

            # ------------------------- up conv ------------------------------
            wu_t = wpool.tile([128, 40 * 128], dt.bfloat16, tag="w")
            _dma(nc.sync.dma_start(wu_t[:, 0:10 * 128], wup[:]))
            for c in range(NCH):
                e0 = c * EC
                # rm1[:, 256:512] = fdrm rows (DRAM->DRAM bounce, interleaved)
                for g in range(EC // 128):
                    eg = e0 + g * 128
                    bt = rpool.tile([128, CO], dt.bfloat16, tag="fdb")
                    _dma(nc.sync.dma_start(bt[:], fdrm[eg:eg + 128, :]))
                    _dma(nc.sync.dma_start(rm1[eg:eg + 128, CO:2 * CO], bt[:]))
                nb = gpool.tile([128, 4, EC], dt.bfloat16, tag="gd0")
                _dma(nc.sync.dma_start(nb[:], nbup[:, :, e0:e0 + EC]))
                ct = cpool.tile([128, EC], dt.bfloat16, tag="ctu")
                _dma(nc.sync.dma_start(ct[:], fu[:, e0:e0 + EC]))
                # features in place: d=a-b -> slot b ; a'=2a-d -> slot a ; |d|
                nc.vector.tensor_tensor(nb[:, 2, :], nb[:, 0, :], nb[:, 2, :],
                                        op=Alu.subtract)
                nc.vector.tensor_tensor(nb[:, 3, :], nb[:, 1, :], nb[:, 3, :],
                                        op=Alu.subtract)
                nc.vector.scalar_tensor_tensor(
                    nb[:, 0, :], nb[:, 0, :], 2.0, nb[:, 2, :],
                    op0=Alu.mult, op1=Alu.subtract)
                nc.vector.scalar_tensor_tensor(
                    nb[:, 1, :], nb[:, 1, :], 2.0, nb[:, 3, :],
                    op0=Alu.mult, op1=Alu.subtract)
                for s in (2, 3):
                    di = nb[:, s, :].bitcast(dt.int16)
                    nc.vector.tensor_scalar(di, di, 0x7FFF, None,
                                            op0=Alu.bitwise_and)
                for ob in range(OB):
                    ps = mmps.tile([128, EC], dt.float32, tag="ps")
                    rhs_by_k = [ct[:], nb[:, 0, :], nb[:, 1, :],
                                nb[:, 2, :], nb[:, 3, :]]
                    for k in range(5):
                        n = ob * 5 + k
                        nc.tensor.matmul(
                            ps[:], wu_t[:, n * 128:(n + 1) * 128], rhs_by_k[k],
                            start=(k == 0), stop=(k == 4))
                    nc.scalar.activation(
                        bufB[:, ob * E + e0:ob * E + e0 + EC], ps[:],
                        Act.Identity, bias=bias_t[:, ob:ob + 1])
                transpose_rows(bufB, e0, rm1, 0, OB)

            # ------------------------ conv pass A ---------------------------
            def conv_pass_a(CB, table, table_c, w_dram, nblk, bias_col,
                            center_fn, raw_dst, stats):
                w_t = wpool.tile([128, 40 * 128], dt.bfloat16, tag="w")
                _dma(nc.sync.dma_start(w_t[:, 0:nblk * 128], w_dram[:]))
                for c in range(NCH):
                    e0 = c * EC
                    gts = []
                    ginsts = []
                    for g in range(4):
                        gt = gpool.tile([128, CB, GNI], dt.bfloat16,
                                        tag=f"gd{g}")
                        ginsts.append(nc.gpsimd.dma_gather(
                            gt[:], table[:],
                            ix_all[:, c * 128 + g * (GNI // 16):
                                   c * 128 + (g + 1) * (GNI // 16)],
                            num_idxs=GNI, num_idxs_reg=GNI,
                            elem_size=table_c, transpose=True, queue_num=g,
                        ))
                        gts.append(gt)
                    _gather_group(ginsts)
                    # features in place (slot-major: 0,1 = n1,n2; 2,3 = n3,n4)
                    nc.vector.tensor_tensor(gts[2][:], gts[0][:], gts[2][:],
                                            op=Alu.subtract)
                    nc.vector.tensor_tensor(gts[3][:], gts[1][:], gts[3][:],
                                            op=Alu.subtract)
                    nc.vector.scalar_tensor_tensor(
                        gts[0][:], gts[0][:], 2.0, gts[2][:],
                        op0=Alu.mult, op1=Alu.subtract)
                    nc.vector.scalar_tensor_tensor(
                        gts[1][:], gts[1][:], 2.0, gts[3][:],
                        op0=Alu.mult, op1=Alu.subtract)
                    for s in (2, 3):
                        di = gts[s][:].bitcast(dt.int16)
                        nc.vector.tensor_scalar(di, di, 0x7FFF, None,
                                                op0=Alu.bitwise_and)
                    ct = center_fn(c)
                    for ob in range(OB):
                        ps = mmps.tile([128, EC], dt.float32, tag="ps")
                        nmm = 5 * CB
                        i_mm = 0
                        for k in range(5):
                            for cb in range(CB):
                                if k == 0:
                                    rhs = ct(cb)
                                else:
                                    rhs = gts[k - 1][:, cb, :]
                                n = (ob * 5 + k) * CB + cb
                                nc.tensor.matmul(
                                    ps[:], w_t[:, n * 128:(n + 1) * 128], rhs,
                                    start=(i_mm == 0), stop=(i_mm == nmm - 1),
                                )
                                i_mm += 1
                        bias_ap = bias_t[:, bias_col * OB + ob:
                                         bias_col * OB + ob + 1]
                        raw_ap = raw_dst[:, ob * E + e0:ob * E + e0 + EC]
                        if stats:
                            nc.scalar.activation(
                                raw_ap, ps[:], Act.Identity, bias=bias_ap,
                                accum_out=ssum[:, ob * NCH + c:ob * NCH + c + 1],
                            )
                            jk = jkpool.tile([128, EC], dt.bfloat16, tag="jk")
                            nc.vector.scalar_tensor_tensor(
                                jk[:], raw_ap, 1.0, raw_ap,
                                op0=Alu.mult, op1=Alu.mult,
                                accum_out=ssq[:, ob * NCH + c:ob * NCH + c + 1],
                            )
                        else:
                            nc.scalar.activation(
                                raw_ap, ps[:], Act.Identity, bias=bias_ap,
                            )

            # -------------------- stats finalize ---------------------------
            def conv_finalize(slot):
                mean = nrm[:, 0:OB]
                var = nrm[:, OB:2 * OB]
                scal = nrm[:, (2 + 2 * slot) * OB:(3 + 2 * slot) * OB]
                shift = nrm[:, (3 + 2 * slot) * OB:(4 + 2 * slot) * OB]
                for ob in range(OB):
                    nc.vector.reduce_sum(
                        mean[:, ob:ob + 1], ssum[:, ob * NCH:(ob + 1) * NCH],
                        axis=mybir.AxisListType.X)
                    nc.vector.reduce_sum(
                        var[:, ob:ob + 1], ssq[:, ob * NCH:(ob + 1) * NCH],
                        axis=mybir.AxisListType.X)
                nc.vector.tensor_scalar(mean, mean, 1.0 / E, None, op0=Alu.mult)
                nc.vector.tensor_scalar(var, var, 1.0 / E, None, op0=Alu.mult)
                nc.vector.scalar_tensor_tensor(
                    shift, mean, -1.0, mean, op0=Alu.mult, op1=Alu.mult)
                nc.vector.tensor_tensor(var, var, shift, op=Alu.add)
                nc.vector.tensor_scalar(var, var, EPS, None, op0=Alu.add)
                nc.scalar.activation(var, var, Act.Sqrt)
                nc.vector.reciprocal(scal, var)
                nc.vector.scalar_tensor_tensor(
                    shift, mean, -1.0, scal, op0=Alu.mult, op1=Alu.mult)
                return scal, shift

            # ------------------------- conv1 --------------------------------
            def c1_center(c):
                e0 = c * EC
                ctf = cpool.tile([128, 2, EC], dt.bfloat16, tag="ctf")
                _dma(nc.sync.dma_start(ctf[:], fd[:, :, e0:e0 + EC]))

                def get(cb):
                    if cb < 2:
                        return bufB[:, cb * E + e0:cb * E + e0 + EC]
                    return ctf[:, cb - 2, :]
                return get

            conv_pass_a(4, rm1, 2 * CO, w1, 40, 1, c1_center, bufA, stats=True)

            # c1B: x1n = relu(norm(raw1)) -> bufB ; transposes -> rm2
            scal, shift = conv_finalize(0)
            for c in range(NCH):
                e0 = c * EC
                for ob in range(OB):
                    nc.scalar.activation(
                        bufB[:, ob * E + e0:ob * E + e0 + EC],
                        bufA[:, ob * E + e0:ob * E + e0 + EC],
                        Act.Relu, bias=shift[:, ob:ob + 1],
                        scale=scal[:, ob:ob + 1])
                transpose_rows(bufB, e0, rm2, 0, OB)

            # ------------------------- conv2a -------------------------------
            def c2_center(c):
                e0 = c * EC
                return lambda cb: bufB[:, cb * E + e0:cb * E + e0 + EC]

            conv_pass_a(2, rm2, CO, w2a, 20, 2, c2_center, bufA, stats=True)

            # c2aB: x2 = relu(norm(raw2a) + x1n) -> bufB ; transposes -> rm3
            scal, shift = conv_finalize(1)
            for c in range(NCH):
                e0 = c * EC
                for ob in range(OB):
                    t = bpool.tile([128, EC], dt.bfloat16, tag="bt")
                    nc.scalar.activation(
                        t[:], bufA[:, ob * E + e0:ob * E + e0 + EC],
                        Act.Identity, bias=shift[:, ob:ob + 1],
                        scale=scal[:, ob:ob + 1])
                    nc.gpsimd.tensor_tensor(
                        t[:], t[:], bufB[:, ob * E + e0:ob * E + e0 + EC],
                        op=Alu.add)
                    nc.gpsimd.tensor_scalar(
                        bufB[:, ob * E + e0:ob * E + e0 + EC], t[:], 0.0, None,
                        op0=Alu.max)
                transpose_rows(bufB, e0, rm3, 0, OB)

            # ------------------------- conv2b -------------------------------
            conv_pass_a(2, rm3, CO, w2b, 20, 3, c2_center, bufA, stats=True)

            # c2bB: out = relu(norm(raw2b) + x2) -> DRAM f32
            scal, shift = conv_finalize(2)
            for c in range(NCH):
                e0 = c * EC
                for ob in range(OB):
                    t = bpool.tile([128, EC], dt.bfloat16, tag="bt")
                    nc.scalar.activation(
                        t[:], bufA[:, ob * E + e0:ob * E + e0 + EC],
                        Act.Identity, bias=shift[:, ob:ob + 1],
                        scale=scal[:, ob:ob + 1])
                    u = bpool.tile([128, EC], dt.float32, tag="ut")
                    nc.gpsimd.tensor_tensor(
                        u[:], t[:], bufB[:, ob * E + e0:ob * E + e0 + EC],
                        op=Alu.add)
                    nc.gpsimd.tensor_scalar(u[:], u[:], 0.0, None, op0=Alu.max)
                    _dma(nc.sync.dma_start(
                        out[ob * 128:(ob + 1) * 128, e0:e0 + EC], u[:]))

    nc.finalize()
    return nc


_NC_CACHE = {}


def _get_nc(E):
    if E not in _NC_CACHE:
        _NC_CACHE[E] = build_nc(E)
    return _NC_CACHE[E]


def make_in_maps(from_up, from_down, edge_index, W_up, b_up, W1, b1, W2a, b2a,
                 W2b, b2b, E=E_FULL):
    """Build the per-core input maps (host-side sharding + layout packing)."""
    wup_p = _pack_w(np.asarray(W_up))
    w1_p = _pack_w(np.asarray(W1))
    w2a_p = _pack_w(np.asarray(W2a))
    w2b_p = _pack_w(np.asarray(W2b))
    bia_p = np.concatenate(
        [_pack_b(b_up), _pack_b(b1), _pack_b(b2a), _pack_b(b2b)], axis=1)
    ident = np.eye(128, dtype=BF16)
    in_maps = []
    for i in range(B):
        fu_b = np.asarray(from_up[i], np.float32).astype(BF16)       # [128,E]
        fd_b = np.asarray(from_down[i], np.float32).astype(BF16)     # [256,E]
        ei = np.asarray(edge_index[i])                               # [E,4]
        nb = np.stack([fu_b[:, ei[:, s]] for s in range(4)], axis=1)  # [128,4,E]
        in_maps.append({
            "nbup": np.ascontiguousarray(nb),
            "fu": fu_b,
            "fd": np.ascontiguousarray(
                fd_b.reshape(2, 128, E).transpose(1, 0, 2)),
            "fdrm": np.ascontiguousarray(fd_b.T),
            "idx": _pack_idx(ei, E),
            "wup": wup_p, "w1": w1_p, "w2a": w2a_p, "w2b": w2b_p,
            "bia": bia_p, "ident": ident,
        })
    return in_maps


def kernel(from_up, from_down, edge_index, W_up, b_up, W1, b1, W2a, b2a,
           W2b, b2b) -> np.ndarray:
    from concourse import bass_utils

    nc = _get_nc(E_FULL)
    in_maps = make_in_maps(from_up, from_down, edge_index, W_up, b_up,
                           W1, b1, W2a, b2a, W2b, b2b)
    res = bass_utils.run_bass_kernel_spmd(nc, in_maps, core_ids=list(range(B)))
    return np.stack([r["out"] for r in res.results]).astype(np.float32)


# revision 22
# speedup vs baseline: 1.4453x; 1.3527x over previous
"""Trainium2 Bass kernel for nn_MeshUpConv (MeshCNN up-conv block).

Strategy: data-parallel over batch B=8 (one mesh per NeuronCore).

v2 changes vs baseline:
  - up-conv neighbor features are host-gathered (pure input rearrangement)
    and shipped channel-major in bf16 -> no on-device gathers for up conv,
    no fu_rm table, no input-transpose passes.
  - from_down shipped pre-cast bf16 both channel-major (centers) and
    row-major (rm1 fd-half, copied DRAM->DRAM via SBUF bounce).
  - remaining dma_gathers (conv1/conv2a/conv2b) spread across 4 SWDGE
    queues -> descriptor generation runs on 4 Q7 core pairs in parallel
    (it was the serialized bottleneck: ~9ns/row on one pair).
  - all idx chunks prefetched once (idx identical across convs).
  - feature build writes sums/absdiffs in place into the gather tiles
    (d = a-b; a' = 2a-d; |d|), no separate diff tiles.
  - plain DMAs on sync engine; gather<->plain fence kept (xbar hazard),
    per-queue gather chaining only.
"""

import sys

for _p in ("/opt/trn_rl_repo",):
    if _p not in sys.path:
        sys.path.append(_p)

import numpy as np
import ml_dtypes

BF16 = ml_dtypes.bfloat16

B = 8
E_FULL = 16384
CIN = 128
CO = 256
OB = 2          # output channel blocks of 128
EC = 512        # edges per chunk
EPS = 1e-5
GNI = 512       # idxs per dma_gather (ring-size limited)
NQ = 4          # SWDGE queues
SAFE_CHAIN = True   # chain all gathers globally (baseline hazard discipline)
PREP_TRIGGER = False  # prepare on 4 queues (parallel desc-gen), serialize fires


def _pack_idx(ei: np.ndarray, E: int) -> np.ndarray:
    """ei [E,4] int32 -> [128, NCH*128] int16 wrapped gather-index layout.

    Per chunk c the 2048 indices are ordered j = s*EC + i (slot-major), and
    index j lives at [16*g + j%16, c*128 + j//16] for every g in 0..7.
    """
    nch = E // EC
    arr = ei.reshape(nch, EC, 4).transpose(0, 2, 1).reshape(nch, 4 * EC)
    w = arr.reshape(nch, (4 * EC) // 16, 16).transpose(2, 0, 1).reshape(16, -1)
    return np.tile(w, (8, 1)).astype(np.int16)


def _pack_w(W: np.ndarray) -> np.ndarray:
    """W [256, C, 5] f32 -> [128, NBLK*128] bf16 lhsT blocks ordered (ob,k,cb)."""
    O, C, K = W.shape
    cb_n = C // 128
    out = np.empty((128, OB * K * cb_n * 128), np.float32)
    n = 0
    for ob in range(OB):
        for k in range(K):
            for cb in range(cb_n):
                blk = W[ob * 128:(ob + 1) * 128, cb * 128:(cb + 1) * 128, k].T
                out[:, n * 128:(n + 1) * 128] = blk
                n += 1
    return out.astype(BF16)


def _pack_b(b: np.ndarray) -> np.ndarray:
    return np.asarray(b).reshape(OB, 128).T.astype(np.float32).copy()


def build_nc(E: int = E_FULL):
    import concourse.bacc as bacc
    import concourse.mybir as mybir
    from concourse.tile import TileContext
    from concourse.tile_rust import add_dep_helper

    dt = mybir.dt
    Alu = mybir.AluOpType
    Act = mybir.ActivationFunctionType
    NCH = E // EC

    nc = bacc.Bacc("TRN2", num_swdge_queues=NQ)

    nbup = nc.dram_tensor("nbup", [128, 4, E], dt.bfloat16, kind="ExternalInput")
    fu = nc.dram_tensor("fu", [128, E], dt.bfloat16, kind="ExternalInput")
    fd = nc.dram_tensor("fd", [128, 2, E], dt.bfloat16, kind="ExternalInput")
    fdrm = nc.dram_tensor("fdrm", [E, CO], dt.bfloat16, kind="ExternalInput")
    idx = nc.dram_tensor("idx", [128, NCH * 128], dt.int16, kind="ExternalInput")
    wup = nc.dram_tensor("wup", [128, 10 * 128], dt.bfloat16, kind="ExternalInput")
    w1 = nc.dram_tensor("w1", [128, 40 * 128], dt.bfloat16, kind="ExternalInput")
    w2a = nc.dram_tensor("w2a", [128, 20 * 128], dt.bfloat16, kind="ExternalInput")
    w2b = nc.dram_tensor("w2b", [128, 20 * 128], dt.bfloat16, kind="ExternalInput")
    bia = nc.dram_tensor("bia", [128, 4 * OB], dt.float32, kind="ExternalInput")
    ident = nc.dram_tensor("ident", [128, 128], dt.bfloat16, kind="ExternalInput")
    out = nc.dram_tensor("out", [CO, E], dt.float32, kind="ExternalOutput")

    rm1 = nc.dram_tensor("rm1", [E, 2 * CO], dt.bfloat16, kind="Internal")
    rm2 = nc.dram_tensor("rm2", [E, CO], dt.bfloat16, kind="Internal")
    rm3 = nc.dram_tensor("rm3", [E, CO], dt.bfloat16, kind="Internal")

    # xbar-transpose gathers deadlock the SDMA engines when a plain DMA copy
    # runs concurrently. Fence: every gather group depends on all plain DMAs
    # issued since the previous group, and every later plain DMA depends on
    # the most recent gather of each queue. Same-queue gathers are ordered by
    # the per-queue ring FIFO; the 4 queues' descriptor generation runs on 4
    # Q7 core pairs in parallel.
    _pending = []
    _last_gather = [None] * NQ

    _wtail = []  # PREP_TRIGGER: last chunk's transfer-complete wait insts

    def _dma(inst):
        if PREP_TRIGGER:
            for w in _wtail:
                add_dep_helper(inst.ins, w.ins,
                               reason="dma-after-gather-fence")
        else:
            for q in range(NQ):
                if _last_gather[q] is not None:
                    add_dep_helper(inst.ins, _last_gather[q].ins,
                                   reason="dma-after-gather-fence")
        _pending.append(inst)
        return inst

    def _gather_group(insts):
        if PREP_TRIGGER:
            return  # fence deps attach to the first trigger instead
        for inst in insts:
            for d in _pending:
                add_dep_helper(inst.ins, d.ins, reason="gather-fence")
        _pending.clear()
        if SAFE_CHAIN:
            prev = _last_gather[0]
            for inst in insts:
                if prev is not None:
                    add_dep_helper(inst.ins, prev.ins, reason="gather-chain")
                prev = inst
            _last_gather[0] = prev
        else:
            for q, inst in enumerate(insts):
                _last_gather[q] = inst

    with TileContext(nc) as tc:
        with (
            tc.tile_pool(name="persist", bufs=1) as persist,
            tc.tile_pool(name="wp", bufs=1) as wpool,
            tc.tile_pool(name="gp", bufs=2) as gpool,
            tc.tile_pool(name="cp", bufs=2) as cpool,
            tc.tile_pool(name="rp", bufs=2) as rpool,
            tc.tile_pool(name="bp", bufs=2) as bpool,
            tc.tile_pool(name="jkp", bufs=1) as jkpool,
            tc.tile_pool(name="mmps", bufs=4, space="PSUM") as mmps,
            tc.tile_pool(name="tpps", bufs=2, space="PSUM") as tpps,
        ):
            bufA = persist.tile([128, OB * E], dt.bfloat16, tag="bufA")
            bufB = persist.tile([128, OB * E], dt.bfloat16, tag="bufB")
            id_t = persist.tile([128, 128], dt.bfloat16, tag="ident")
            bias_t = persist.tile([128, 4 * OB], dt.float32, tag="bias")
            ssum = persist.tile([128, OB * NCH], dt.float32, tag="ssum")
            ssq = persist.tile([128, OB * NCH], dt.float32, tag="ssq")
            nrm = persist.tile([128, 8 * OB], dt.float32, tag="nrm")
            ix_all = persist.tile([128, NCH * 128], dt.int16, tag="ix")

            _dma(nc.sync.dma_start(id_t[:], ident[:]))
            _dma(nc.sync.dma_start(bias_t[:], bia[:]))
            _dma(nc.sync.dma_start(ix_all[:], idx[:]))

            # ---------------- shared epilogue: transposes -> rm -------------
            def transpose_rows(src_buf, e0, rm_dst, col0, ncol_ob):
                """PE-transpose src_buf chunk (both ob) -> rm rows e0..e0+EC."""
                tps = []
                for ob in range(ncol_ob):
                    tp = tpps.tile([128, EC], dt.bfloat16, tag=f"tp{ob}")
                    for g in range(EC // 128):
                        nc.tensor.transpose(
                            tp[:, g * 128:(g + 1) * 128],
                            src_buf[:, ob * E + e0 + g * 128:
                                    ob * E + e0 + g * 128 + 128],
                            id_t[:])
                    tps.append(tp)
                rtc = rpool.tile([128, EC // 128, ncol_ob * 128], dt.bfloat16,
                                 tag="rtc")
                for ob in range(ncol_ob):
                    nc.vector.tensor_copy(
                        rtc[:].rearrange("p g (o n) -> p g o n", o=ncol_ob)
                        [:, :, ob, :],
                        tps[ob][:].rearrange("p (g n) -> p g n", g=EC // 128))
                for g in range(EC // 128):
                    eg = e0 + g * 128
                    _dma(nc.sync.dma_start(
                        rm_dst[eg:eg + 128, col0:col0 + ncol_ob * 128],
                        rtc[:, g, :]))

            # ------------------------- up conv ------------------------------
            wu_t = wpool.tile([128, 40 * 128], dt.bfloat16, tag="w")
            _dma(nc.sync.dma_start(wu_t[:, 0:10 * 128], wup[:]))
            for c in range(NCH):
                e0 = c * EC
                # rm1[:, 256:512] = fdrm rows (DRAM->DRAM bounce, interleaved)
                for g in range(EC // 128):
                    eg = e0 + g * 128
                    bt = rpool.tile([128, CO], dt.bfloat16, tag="fdb")
                    _dma(nc.sync.dma_start(bt[:], fdrm[eg:eg + 128, :]))
                    _dma(nc.sync.dma_start(rm1[eg:eg + 128, CO:2 * CO], bt[:]))
                nb = cpool.tile([128, 4, EC], dt.bfloat16, tag="nbu")
                _dma(nc.sync.dma_start(nb[:], nbup[:, :, e0:e0 + EC]))
                ct = cpool.tile([128, EC], dt.bfloat16, tag="ctu")
                _dma(nc.sync.dma_start(ct[:], fu[:, e0:e0 + EC]))
                # features in place: d=a-b -> slot b ; a'=2a-d -> slot a ; |d|
                nc.vector.tensor_tensor(nb[:, 2, :], nb[:, 0, :], nb[:, 2, :],
                                        op=Alu.subtract)
                nc.vector.tensor_tensor(nb[:, 3, :], nb[:, 1, :], nb[:, 3, :],
                                        op=Alu.subtract)
                nc.vector.scalar_tensor_tensor(
                    nb[:, 0, :], nb[:, 0, :], 2.0, nb[:, 2, :],
                    op0=Alu.mult, op1=Alu.subtract)
                nc.vector.scalar_tensor_tensor(
                    nb[:, 1, :], nb[:, 1, :], 2.0, nb[:, 3, :],
                    op0=Alu.mult, op1=Alu.subtract)
                for s in (2, 3):
                    di = nb[:, s, :].bitcast(dt.int16)
                    nc.vector.tensor_scalar(di, di, 0x7FFF, None,
                                            op0=Alu.bitwise_and)
                for ob in range(OB):
                    ps = mmps.tile([128, EC], dt.float32, tag="ps")
                    rhs_by_k = [ct[:], nb[:, 0, :], nb[:, 1, :],
                                nb[:, 2, :], nb[:, 3, :]]
                    for k in range(5):
                        n = ob * 5 + k
                        nc.tensor.matmul(
                            ps[:], wu_t[:, n * 128:(n + 1) * 128], rhs_by_k[k],
                            start=(k == 0), stop=(k == 4))
                    nc.scalar.activation(
                        bufB[:, ob * E + e0:ob * E + e0 + EC], ps[:],
                        Act.Identity, bias=bias_t[:, ob:ob + 1])
                transpose_rows(bufB, e0, rm1, 0, OB)

            gsems = [nc.alloc_semaphore(f"gdma{q}") for q in range(NQ)] \
                if PREP_TRIGGER else []
            gstate = {"n": 0, "tail": None, "cnt": [0] * NQ, "mm_hist": []}

            # ------------------------ conv pass A ---------------------------
            def conv_pass_a(CB, table, table_c, w_dram, nblk, bias_col,
                            center_fn, raw_dst, stats):
                w_t = wpool.tile([128, 40 * 128], dt.bfloat16, tag="w")
                _dma(nc.sync.dma_start(w_t[:, 0:nblk * 128], w_dram[:]))
                for c in range(NCH):
                    e0 = c * EC
                    gts = []
                    ginsts = []
                    for g in range(4):
                        gt = gpool.tile([128, CB, GNI], dt.bfloat16,
                                        tag=f"gd{g}")
                        ginsts.append(nc.gpsimd.dma_gather(
                            gt[:], table[:],
                            ix_all[:, c * 128 + g * (GNI // 16):
                                   c * 128 + (g + 1) * (GNI // 16)],
                            num_idxs=GNI, num_idxs_reg=GNI,
                            elem_size=table_c, transpose=True, queue_num=g,
                            prepare_only=PREP_TRIGGER,
                            sem=gsems[g] if PREP_TRIGGER else None,
                        ))
                        gts.append(gt)
                    _gather_group(ginsts)
                    dve_waits = []
                    if PREP_TRIGGER:
                        # Serialize the 4 queues' transfers: each trigger
                        # fires only after the previous gather's DMA landed
                        # (the xbar transpose path cannot run concurrently).
                        # The wait chain is transitive across queues.
                        for g in range(4):
                            prev = gstate["tail"]
                            if gstate["n"] > 0:
                                qp = (g + NQ - 1) % NQ
                                w = nc.gpsimd.wait_ge(
                                    gsems[qp], 16 * gstate["cnt"][qp])
                                if prev is not None:
                                    add_dep_helper(w.ins, prev.ins,
                                                   reason="trig-chain")
                                prev = w
                            t = nc.gpsimd.trigger_dma(count=1, queue_num=g)
                            if prev is not None:
                                add_dep_helper(t.ins, prev.ins,
                                               reason="trig-chain")
                            if g == 0:
                                # plain-DMA exclusion + table-write RAW:
                                # the first fire waits for all plains since
                                # the previous chunk's fires.
                                for d in _pending:
                                    add_dep_helper(t.ins, d.ins,
                                                   reason="gather-fence")
                                _pending.clear()
                                # WAR: this transfer overwrites the tiles the
                                # chunk-before-last's matmuls read from.
                                if len(gstate["mm_hist"]) >= 2:
                                    add_dep_helper(
                                        t.ins, gstate["mm_hist"][-2].ins,
                                        reason="gather-war")
                            gstate["tail"] = t
                            gstate["n"] += 1
                            gstate["cnt"][g] += 1
                        # RAW: consumers must wait for the transfers to land.
                        for g in range(4):
                            wv = nc.vector.wait_ge(gsems[g],
                                                   16 * gstate["cnt"][g])
                            dve_waits.append(wv)
                        _wtail.clear()
                        _wtail.extend(dve_waits)
                    # features in place (slot-major: 0,1 = n1,n2; 2,3 = n3,n4)
                    f13 = nc.vector.tensor_tensor(gts[2][:], gts[0][:],
                                                  gts[2][:], op=Alu.subtract)
                    f24 = nc.vector.tensor_tensor(gts[3][:], gts[1][:],
                                                  gts[3][:], op=Alu.subtract)
                    for wv in dve_waits:
                        add_dep_helper(f13.ins, wv.ins, reason="gather-raw")
                        add_dep_helper(f24.ins, wv.ins, reason="gather-raw")
                    nc.vector.scalar_tensor_tensor(
                        gts[0][:], gts[0][:], 2.0, gts[2][:],
                        op0=Alu.mult, op1=Alu.subtract)
                    nc.vector.scalar_tensor_tensor(
                        gts[1][:], gts[1][:], 2.0, gts[3][:],
                        op0=Alu.mult, op1=Alu.subtract)
                    for s in (2, 3):
                        di = gts[s][:].bitcast(dt.int16)
                        nc.vector.tensor_scalar(di, di, 0x7FFF, None,
                                                op0=Alu.bitwise_and)
                    ct = center_fn(c)
                    last_mm = None
                    for ob in range(OB):
                        ps = mmps.tile([128, EC], dt.float32, tag="ps")
                        nmm = 5 * CB
                        i_mm = 0
                        for k in range(5):
                            for cb in range(CB):
                                if k == 0:
                                    rhs = ct(cb)
                                else:
                                    rhs = gts[k - 1][:, cb, :]
                                n = (ob * 5 + k) * CB + cb
                                last_mm = nc.tensor.matmul(
                                    ps[:], w_t[:, n * 128:(n + 1) * 128], rhs,
                                    start=(i_mm == 0), stop=(i_mm == nmm - 1),
                                )
                                i_mm += 1
                        bias_ap = bias_t[:, bias_col * OB + ob:
                                         bias_col * OB + ob + 1]
                        raw_ap = raw_dst[:, ob * E + e0:ob * E + e0 + EC]
                        if stats:
                            nc.scalar.activation(
                                raw_ap, ps[:], Act.Identity, bias=bias_ap,
                                accum_out=ssum[:, ob * NCH + c:ob * NCH + c + 1],
                            )
                            jk = jkpool.tile([128, EC], dt.bfloat16, tag="jk")
                            nc.vector.scalar_tensor_tensor(
                                jk[:], raw_ap, 1.0, raw_ap,
                                op0=Alu.mult, op1=Alu.mult,
                                accum_out=ssq[:, ob * NCH + c:ob * NCH + c + 1],
                            )
                        else:
                            nc.scalar.activation(
                                raw_ap, ps[:], Act.Identity, bias=bias_ap,
                            )
                    gstate["mm_hist"].append(last_mm)

            # -------------------- stats finalize ---------------------------
            def conv_finalize(slot):
                mean = nrm[:, 0:OB]
                var = nrm[:, OB:2 * OB]
                scal = nrm[:, (2 + 2 * slot) * OB:(3 + 2 * slot) * OB]
                shift = nrm[:, (3 + 2 * slot) * OB:(4 + 2 * slot) * OB]
                for ob in range(OB):
                    nc.vector.reduce_sum(
                        mean[:, ob:ob + 1], ssum[:, ob * NCH:(ob + 1) * NCH],
                        axis=mybir.AxisListType.X)
                    nc.vector.reduce_sum(
                        var[:, ob:ob + 1], ssq[:, ob * NCH:(ob + 1) * NCH],
                        axis=mybir.AxisListType.X)
                nc.vector.tensor_scalar(mean, mean, 1.0 / E, None, op0=Alu.mult)
                nc.vector.tensor_scalar(var, var, 1.0 / E, None, op0=Alu.mult)
                nc.vector.scalar_tensor_tensor(
                    shift, mean, -1.0, mean, op0=Alu.mult, op1=Alu.mult)
                nc.vector.tensor_tensor(var, var, shift, op=Alu.add)
                nc.vector.tensor_scalar(var, var, EPS, None, op0=Alu.add)
                nc.scalar.activation(var, var, Act.Sqrt)
                nc.vector.reciprocal(scal, var)
                nc.vector.scalar_tensor_tensor(
                    shift, mean, -1.0, scal, op0=Alu.mult, op1=Alu.mult)
                return scal, shift

            # ------------------------- conv1 --------------------------------
            def c1_center(c):
                e0 = c * EC
                ctf = cpool.tile([128, 2, EC], dt.bfloat16, tag="ctf")
                _dma(nc.sync.dma_start(ctf[:], fd[:, :, e0:e0 + EC]))

                def get(cb):
                    if cb < 2:
                        return bufB[:, cb * E + e0:cb * E + e0 + EC]
                    return ctf[:, cb - 2, :]
                return get

            conv_pass_a(4, rm1, 2 * CO, w1, 40, 1, c1_center, bufA, stats=True)

            # c1B: x1n = relu(norm(raw1)) -> bufB ; transposes -> rm2
            scal, shift = conv_finalize(0)
            for c in range(NCH):
                e0 = c * EC
                for ob in range(OB):
                    nc.scalar.activation(
                        bufB[:, ob * E + e0:ob * E + e0 + EC],
                        bufA[:, ob * E + e0:ob * E + e0 + EC],
                        Act.Relu, bias=shift[:, ob:ob + 1],
                        scale=scal[:, ob:ob + 1])
                transpose_rows(bufB, e0, rm2, 0, OB)

            # ------------------------- conv2a -------------------------------
            def c2_center(c):
                e0 = c * EC
                return lambda cb: bufB[:, cb * E + e0:cb * E + e0 + EC]

            conv_pass_a(2, rm2, CO, w2a, 20, 2, c2_center, bufA, stats=True)

            # c2aB: x2 = relu(norm(raw2a) + x1n) -> bufB ; transposes -> rm3
            scal, shift = conv_finalize(1)
            for c in range(NCH):
                e0 = c * EC
                for ob in range(OB):
                    t = bpool.tile([128, EC], dt.bfloat16, tag="bt")
                    nc.scalar.activation(
                        t[:], bufA[:, ob * E + e0:ob * E + e0 + EC],
                        Act.Identity, bias=shift[:, ob:ob + 1],
                        scale=scal[:, ob:ob + 1])
                    nc.gpsimd.tensor_tensor(
                        t[:], t[:], bufB[:, ob * E + e0:ob * E + e0 + EC],
                        op=Alu.add)
                    nc.gpsimd.tensor_scalar(
                        bufB[:, ob * E + e0:ob * E + e0 + EC], t[:], 0.0, None,
                        op0=Alu.max)
                transpose_rows(bufB, e0, rm3, 0, OB)

            # ------------------------- conv2b -------------------------------
            conv_pass_a(2, rm3, CO, w2b, 20, 3, c2_center, bufA, stats=True)

            # c2bB: out = relu(norm(raw2b) + x2) -> DRAM f32
            scal, shift = conv_finalize(2)
            for c in range(NCH):
                e0 = c * EC
                for ob in range(OB):
                    t = bpool.tile([128, EC], dt.bfloat16, tag="bt")
                    nc.scalar.activation(
                        t[:], bufA[:, ob * E + e0:ob * E + e0 + EC],
                        Act.Identity, bias=shift[:, ob:ob + 1],
                        scale=scal[:, ob:ob + 1])
                    u = bpool.tile([128, EC], dt.float32, tag="ut")
                    nc.gpsimd.tensor_tensor(
                        u[:], t[:], bufB[:, ob * E + e0:ob * E + e0 + EC],
                        op=Alu.add)
                    nc.gpsimd.tensor_scalar(u[:], u[:], 0.0, None, op0=Alu.max)
                    _dma(nc.sync.dma_start(
                        out[ob * 128:(ob + 1) * 128, e0:e0 + EC], u[:]))

    nc.finalize()
    return nc


_NC_CACHE = {}


def _get_nc(E):
    if E not in _NC_CACHE:
        _NC_CACHE[E] = build_nc(E)
    return _NC_CACHE[E]


def make_in_maps(from_up, from_down, edge_index, W_up, b_up, W1, b1, W2a, b2a,
                 W2b, b2b, E=E_FULL):
    """Build the per-core input maps (host-side sharding + layout packing)."""
    wup_p = _pack_w(np.asarray(W_up))
    w1_p = _pack_w(np.asarray(W1))
    w2a_p = _pack_w(np.asarray(W2a))
    w2b_p = _pack_w(np.asarray(W2b))
    bia_p = np.concatenate(
        [_pack_b(b_up), _pack_b(b1), _pack_b(b2a), _pack_b(b2b)], axis=1)
    ident = np.eye(128, dtype=BF16)
    in_maps = []
    for i in range(B):
        fu_b = np.asarray(from_up[i], np.float32).astype(BF16)       # [128,E]
        fd_b = np.asarray(from_down[i], np.float32).astype(BF16)     # [256,E]
        ei = np.asarray(edge_index[i])                               # [E,4]
        nb = np.stack([fu_b[:, ei[:, s]] for s in range(4)], axis=1)  # [128,4,E]
        in_maps.append({
            "nbup": np.ascontiguousarray(nb),
            "fu": fu_b,
            "fd": np.ascontiguousarray(
                fd_b.reshape(2, 128, E).transpose(1, 0, 2)),
            "fdrm": np.ascontiguousarray(fd_b.T),
            "idx": _pack_idx(ei, E),
            "wup": wup_p, "w1": w1_p, "w2a": w2a_p, "w2b": w2b_p,
            "bia": bia_p, "ident": ident,
        })
    return in_maps


def kernel(from_up, from_down, edge_index, W_up, b_up, W1, b1, W2a, b2a,
           W2b, b2b) -> np.ndarray:
    from concourse import bass_utils

    nc = _get_nc(E_FULL)
    in_maps = make_in_maps(from_up, from_down, edge_index, W_up, b_up,
                           W1, b1, W2a, b2a, W2b, b2b)
    res = bass_utils.run_bass_kernel_spmd(nc, in_maps, core_ids=list(range(B)))
    return np.stack([r["out"] for r in res.results]).astype(np.float32)


# revision 23
# speedup vs baseline: 1.8800x; 1.3007x over previous
"""Trainium2 Bass kernel for nn_MeshUpConv (MeshCNN up-conv block).

Strategy: data-parallel over batch B=8 (one mesh per NeuronCore).

v2 changes vs baseline:
  - up-conv neighbor features are host-gathered (pure input rearrangement)
    and shipped channel-major in bf16 -> no on-device gathers for up conv,
    no fu_rm table, no input-transpose passes.
  - from_down shipped pre-cast bf16 both channel-major (centers) and
    row-major (rm1 fd-half, copied DRAM->DRAM via SBUF bounce).
  - remaining dma_gathers (conv1/conv2a/conv2b) spread across 4 SWDGE
    queues -> descriptor generation runs on 4 Q7 core pairs in parallel
    (it was the serialized bottleneck: ~9ns/row on one pair).
  - all idx chunks prefetched once (idx identical across convs).
  - feature build writes sums/absdiffs in place into the gather tiles
    (d = a-b; a' = 2a-d; |d|), no separate diff tiles.
  - plain DMAs on sync engine; gather<->plain fence kept (xbar hazard),
    per-queue gather chaining only.
"""

import sys

for _p in ("/opt/trn_rl_repo",):
    if _p not in sys.path:
        sys.path.append(_p)

import numpy as np
import ml_dtypes

BF16 = ml_dtypes.bfloat16

B = 8
E_FULL = 16384
CIN = 128
CO = 256
OB = 2          # output channel blocks of 128
EC = 512        # edges per chunk
EPS = 1e-5
GNI = 512       # idxs per dma_gather (ring-size limited)
NQ = 4          # SWDGE queues
SAFE_CHAIN = False  # chain all gathers globally (baseline hazard discipline)
PREP_TRIGGER = True  # prepare on 4 queues (parallel desc-gen), serialize fires


def _pack_idx(ei: np.ndarray, E: int) -> np.ndarray:
    """ei [E,4] int32 -> [128, NCH*128] int16 wrapped gather-index layout.

    Per chunk c the 2048 indices are ordered j = s*EC + i (slot-major), and
    index j lives at [16*g + j%16, c*128 + j//16] for every g in 0..7.
    """
    nch = E // EC
    arr = ei.reshape(nch, EC, 4).transpose(0, 2, 1).reshape(nch, 4 * EC)
    w = arr.reshape(nch, (4 * EC) // 16, 16).transpose(2, 0, 1).reshape(16, -1)
    return np.tile(w, (8, 1)).astype(np.int16)


def _pack_w(W: np.ndarray) -> np.ndarray:
    """W [256, C, 5] f32 -> [128, NBLK*128] bf16 lhsT blocks ordered (ob,k,cb)."""
    O, C, K = W.shape
    cb_n = C // 128
    out = np.empty((128, OB * K * cb_n * 128), np.float32)
    n = 0
    for ob in range(OB):
        for k in range(K):
            for cb in range(cb_n):
                blk = W[ob * 128:(ob + 1) * 128, cb * 128:(cb + 1) * 128, k].T
                out[:, n * 128:(n + 1) * 128] = blk
                n += 1
    return out.astype(BF16)


def _pack_b(b: np.ndarray) -> np.ndarray:
    return np.asarray(b).reshape(OB, 128).T.astype(np.float32).copy()


def build_nc(E: int = E_FULL):
    import concourse.bacc as bacc
    import concourse.mybir as mybir
    from concourse.tile import TileContext
    from concourse.tile_rust import add_dep_helper

    dt = mybir.dt
    Alu = mybir.AluOpType
    Act = mybir.ActivationFunctionType
    NCH = E // EC

    nc = bacc.Bacc("TRN2", num_swdge_queues=NQ)

    nbup = nc.dram_tensor("nbup", [128, 4, E], dt.bfloat16, kind="ExternalInput")
    fu = nc.dram_tensor("fu", [128, E], dt.bfloat16, kind="ExternalInput")
    fd = nc.dram_tensor("fd", [128, 2, E], dt.bfloat16, kind="ExternalInput")
    fdrm = nc.dram_tensor("fdrm", [E, CO], dt.bfloat16, kind="ExternalInput")
    idx = nc.dram_tensor("idx", [128, NCH * 128], dt.int16, kind="ExternalInput")
    wup = nc.dram_tensor("wup", [128, 10 * 128], dt.bfloat16, kind="ExternalInput")
    w1 = nc.dram_tensor("w1", [128, 40 * 128], dt.bfloat16, kind="ExternalInput")
    w2a = nc.dram_tensor("w2a", [128, 20 * 128], dt.bfloat16, kind="ExternalInput")
    w2b = nc.dram_tensor("w2b", [128, 20 * 128], dt.bfloat16, kind="ExternalInput")
    bia = nc.dram_tensor("bia", [128, 4 * OB], dt.float32, kind="ExternalInput")
    ident = nc.dram_tensor("ident", [128, 128], dt.bfloat16, kind="ExternalInput")
    out = nc.dram_tensor("out", [CO, E], dt.float32, kind="ExternalOutput")

    rm1 = nc.dram_tensor("rm1", [E, 2 * CO], dt.bfloat16, kind="Internal")
    rm2 = nc.dram_tensor("rm2", [E, CO], dt.bfloat16, kind="Internal")
    rm3 = nc.dram_tensor("rm3", [E, CO], dt.bfloat16, kind="Internal")

    # xbar-transpose gathers deadlock the SDMA engines when a plain DMA copy
    # runs concurrently. Fence: every gather group depends on all plain DMAs
    # issued since the previous group, and every later plain DMA depends on
    # the most recent gather of each queue. Same-queue gathers are ordered by
    # the per-queue ring FIFO; the 4 queues' descriptor generation runs on 4
    # Q7 core pairs in parallel.
    _pending = []
    _last_gather = [None] * NQ

    _wtail = []  # PREP_TRIGGER: last chunk's transfer-complete wait insts

    def _dma(inst):
        if PREP_TRIGGER:
            for w in _wtail:
                add_dep_helper(inst.ins, w.ins,
                               reason="dma-after-gather-fence")
        else:
            for q in range(NQ):
                if _last_gather[q] is not None:
                    add_dep_helper(inst.ins, _last_gather[q].ins,
                                   reason="dma-after-gather-fence")
        _pending.append(inst)
        return inst

    def _gather_group(insts):
        if PREP_TRIGGER:
            return  # fence deps attach to the first trigger instead
        for inst in insts:
            for d in _pending:
                add_dep_helper(inst.ins, d.ins, reason="gather-fence")
        _pending.clear()
        if SAFE_CHAIN:
            prev = _last_gather[0]
            for inst in insts:
                if prev is not None:
                    add_dep_helper(inst.ins, prev.ins, reason="gather-chain")
                prev = inst
            _last_gather[0] = prev
        else:
            for q, inst in enumerate(insts):
                _last_gather[q] = inst

    with TileContext(nc) as tc:
        with (
            tc.tile_pool(name="persist", bufs=1) as persist,
            tc.tile_pool(name="wp", bufs=1) as wpool,
            tc.tile_pool(name="gp", bufs=2) as gpool,
            tc.tile_pool(name="cp", bufs=2) as cpool,
            tc.tile_pool(name="rp", bufs=2) as rpool,
            tc.tile_pool(name="bp", bufs=2) as bpool,
            tc.tile_pool(name="jkp", bufs=1) as jkpool,
            tc.tile_pool(name="mmps", bufs=4, space="PSUM") as mmps,
            tc.tile_pool(name="tpps", bufs=2, space="PSUM") as tpps,
        ):
            bufA = persist.tile([128, OB * E], dt.bfloat16, tag="bufA")
            bufB = persist.tile([128, OB * E], dt.bfloat16, tag="bufB")
            id_t = persist.tile([128, 128], dt.bfloat16, tag="ident")
            bias_t = persist.tile([128, 4 * OB], dt.float32, tag="bias")
            ssum = persist.tile([128, OB * NCH], dt.float32, tag="ssum")
            ssq = persist.tile([128, OB * NCH], dt.float32, tag="ssq")
            nrm = persist.tile([128, 8 * OB], dt.float32, tag="nrm")
            ix_all = persist.tile([128, NCH * 128], dt.int16, tag="ix")

            _dma(nc.sync.dma_start(id_t[:], ident[:]))
            _dma(nc.sync.dma_start(bias_t[:], bia[:]))
            _dma(nc.sync.dma_start(ix_all[:], idx[:]))

            # ---------------- shared epilogue: transposes -> rm -------------
            def transpose_rows(src_buf, e0, rm_dst, col0, ncol_ob):
                """PE-transpose src_buf chunk (both ob) -> rm rows e0..e0+EC."""
                tps = []
                for ob in range(ncol_ob):
                    tp = tpps.tile([128, EC], dt.bfloat16, tag=f"tp{ob}")
                    for g in range(EC // 128):
                        nc.tensor.transpose(
                            tp[:, g * 128:(g + 1) * 128],
                            src_buf[:, ob * E + e0 + g * 128:
                                    ob * E + e0 + g * 128 + 128],
                            id_t[:])
                    tps.append(tp)
                rtc = rpool.tile([128, EC // 128, ncol_ob * 128], dt.bfloat16,
                                 tag="rtc")
                for ob in range(ncol_ob):
                    nc.vector.tensor_copy(
                        rtc[:].rearrange("p g (o n) -> p g o n", o=ncol_ob)
                        [:, :, ob, :],
                        tps[ob][:].rearrange("p (g n) -> p g n", g=EC // 128))
                for g in range(EC // 128):
                    eg = e0 + g * 128
                    _dma(nc.sync.dma_start(
                        rm_dst[eg:eg + 128, col0:col0 + ncol_ob * 128],
                        rtc[:, g, :]))

            # ------------------------- up conv ------------------------------
            wu_t = wpool.tile([128, 40 * 128], dt.bfloat16, tag="w")
            _dma(nc.sync.dma_start(wu_t[:, 0:10 * 128], wup[:]))
            for c in range(NCH):
                e0 = c * EC
                # rm1[:, 256:512] = fdrm rows (DRAM->DRAM bounce, interleaved)
                for g in range(EC // 128):
                    eg = e0 + g * 128
                    bt = rpool.tile([128, CO], dt.bfloat16, tag="fdb")
                    _dma(nc.sync.dma_start(bt[:], fdrm[eg:eg + 128, :]))
                    _dma(nc.sync.dma_start(rm1[eg:eg + 128, CO:2 * CO], bt[:]))
                nb = cpool.tile([128, 4, EC], dt.bfloat16, tag="nbu")
                _dma(nc.sync.dma_start(nb[:], nbup[:, :, e0:e0 + EC]))
                ct = cpool.tile([128, EC], dt.bfloat16, tag="ctu")
                _dma(nc.sync.dma_start(ct[:], fu[:, e0:e0 + EC]))
                # features in place: d=a-b -> slot b ; a'=2a-d -> slot a ; |d|
                nc.vector.tensor_tensor(nb[:, 2, :], nb[:, 0, :], nb[:, 2, :],
                                        op=Alu.subtract)
                nc.vector.tensor_tensor(nb[:, 3, :], nb[:, 1, :], nb[:, 3, :],
                                        op=Alu.subtract)
                nc.vector.scalar_tensor_tensor(
                    nb[:, 0, :], nb[:, 0, :], 2.0, nb[:, 2, :],
                    op0=Alu.mult, op1=Alu.subtract)
                nc.vector.scalar_tensor_tensor(
                    nb[:, 1, :], nb[:, 1, :], 2.0, nb[:, 3, :],
                    op0=Alu.mult, op1=Alu.subtract)
                for s in (2, 3):
                    di = nb[:, s, :].bitcast(dt.int16)
                    nc.vector.tensor_scalar(di, di, 0x7FFF, None,
                                            op0=Alu.bitwise_and)
                for ob in range(OB):
                    ps = mmps.tile([128, EC], dt.float32, tag="ps")
                    rhs_by_k = [ct[:], nb[:, 0, :], nb[:, 1, :],
                                nb[:, 2, :], nb[:, 3, :]]
                    for k in range(5):
                        n = ob * 5 + k
                        nc.tensor.matmul(
                            ps[:], wu_t[:, n * 128:(n + 1) * 128], rhs_by_k[k],
                            start=(k == 0), stop=(k == 4))
                    nc.scalar.activation(
                        bufB[:, ob * E + e0:ob * E + e0 + EC], ps[:],
                        Act.Identity, bias=bias_t[:, ob:ob + 1])
                transpose_rows(bufB, e0, rm1, 0, OB)

            gsems = [nc.alloc_semaphore(f"gdma{q}") for q in range(NQ)] \
                if PREP_TRIGGER else []
            gstate = {"n": 0, "tail": None, "cnt": [0] * NQ, "mm_hist": []}

            # ------------------------ conv pass A ---------------------------
            def conv_pass_a(CB, table, table_c, w_dram, nblk, bias_col,
                            center_fn, raw_dst, stats):
                w_t = wpool.tile([128, 40 * 128], dt.bfloat16, tag="w")
                _dma(nc.sync.dma_start(w_t[:, 0:nblk * 128], w_dram[:]))
                for c in range(NCH):
                    e0 = c * EC
                    gts = []
                    ginsts = []
                    for g in range(4):
                        gt = gpool.tile([128, CB, GNI], dt.bfloat16,
                                        tag=f"gd{g}")
                        ginsts.append(nc.gpsimd.dma_gather(
                            gt[:], table[:],
                            ix_all[:, c * 128 + g * (GNI // 16):
                                   c * 128 + (g + 1) * (GNI // 16)],
                            num_idxs=GNI, num_idxs_reg=GNI,
                            elem_size=table_c, transpose=True, queue_num=g,
                            prepare_only=PREP_TRIGGER,
                            sem=gsems[g] if PREP_TRIGGER else None,
                        ))
                        gts.append(gt)
                    _gather_group(ginsts)
                    dve_waits = []
                    if PREP_TRIGGER:
                        # Serialize the 4 queues' transfers: each trigger
                        # fires only after the previous gather's DMA landed
                        # (the xbar transpose path cannot run concurrently).
                        # The wait chain is transitive across queues.
                        for g in range(4):
                            prev = gstate["tail"]
                            if gstate["n"] > 0:
                                qp = (g + NQ - 1) % NQ
                                w = nc.gpsimd.wait_ge(
                                    gsems[qp], 16 * gstate["cnt"][qp])
                                if prev is not None:
                                    add_dep_helper(w.ins, prev.ins,
                                                   reason="trig-chain")
                                prev = w
                            t = nc.gpsimd.trigger_dma(count=1, queue_num=g)
                            if prev is not None:
                                add_dep_helper(t.ins, prev.ins,
                                               reason="trig-chain")
                            if g == 0:
                                # plain-DMA exclusion + table-write RAW:
                                # the first fire waits for all plains since
                                # the previous chunk's fires.
                                for d in _pending:
                                    add_dep_helper(t.ins, d.ins,
                                                   reason="gather-fence")
                                _pending.clear()
                                # WAR: this transfer overwrites the tiles the
                                # chunk-before-last's matmuls read from.
                                if len(gstate["mm_hist"]) >= 2:
                                    add_dep_helper(
                                        t.ins, gstate["mm_hist"][-2].ins,
                                        reason="gather-war")
                            gstate["tail"] = t
                            gstate["n"] += 1
                            gstate["cnt"][g] += 1
                        # RAW: consumers must wait for the transfers to land.
                        for g in range(4):
                            wv = nc.vector.wait_ge(gsems[g],
                                                   16 * gstate["cnt"][g])
                            dve_waits.append(wv)
                        _wtail.clear()
                        _wtail.extend(dve_waits)
                    # features in place (slot-major: 0,1 = n1,n2; 2,3 = n3,n4)
                    f13 = nc.vector.tensor_tensor(gts[2][:], gts[0][:],
                                                  gts[2][:], op=Alu.subtract)
                    f24 = nc.vector.tensor_tensor(gts[3][:], gts[1][:],
                                                  gts[3][:], op=Alu.subtract)
                    for wv in dve_waits:
                        add_dep_helper(f13.ins, wv.ins, reason="gather-raw")
                        add_dep_helper(f24.ins, wv.ins, reason="gather-raw")
                    nc.vector.scalar_tensor_tensor(
                        gts[0][:], gts[0][:], 2.0, gts[2][:],
                        op0=Alu.mult, op1=Alu.subtract)
                    nc.vector.scalar_tensor_tensor(
                        gts[1][:], gts[1][:], 2.0, gts[3][:],
                        op0=Alu.mult, op1=Alu.subtract)
                    for s in (2, 3):
                        di = gts[s][:].bitcast(dt.int16)
                        nc.vector.tensor_scalar(di, di, 0x7FFF, None,
                                                op0=Alu.bitwise_and)
                    ct = center_fn(c)
                    last_mm = None
                    for ob in range(OB):
                        ps = mmps.tile([128, EC], dt.float32, tag="ps")
                        nmm = 5 * CB
                        i_mm = 0
                        for k in range(5):
                            for cb in range(CB):
                                if k == 0:
                                    rhs = ct(cb)
                                else:
                                    rhs = gts[k - 1][:, cb, :]
                                n = (ob * 5 + k) * CB + cb
                                last_mm = nc.tensor.matmul(
                                    ps[:], w_t[:, n * 128:(n + 1) * 128], rhs,
                                    start=(i_mm == 0), stop=(i_mm == nmm - 1),
                                )
                                i_mm += 1
                        bias_ap = bias_t[:, bias_col * OB + ob:
                                         bias_col * OB + ob + 1]
                        raw_ap = raw_dst[:, ob * E + e0:ob * E + e0 + EC]
                        if stats:
                            nc.scalar.activation(
                                raw_ap, ps[:], Act.Identity, bias=bias_ap,
                                accum_out=ssum[:, ob * NCH + c:ob * NCH + c + 1],
                            )
                            jk = jkpool.tile([128, EC], dt.bfloat16, tag="jk")
                            nc.vector.scalar_tensor_tensor(
                                jk[:], raw_ap, 1.0, raw_ap,
                                op0=Alu.mult, op1=Alu.mult,
                                accum_out=ssq[:, ob * NCH + c:ob * NCH + c + 1],
                            )
                        else:
                            nc.scalar.activation(
                                raw_ap, ps[:], Act.Identity, bias=bias_ap,
                            )
                    gstate["mm_hist"].append(last_mm)

            # -------------------- stats finalize ---------------------------
            def conv_finalize(slot):
                mean = nrm[:, 0:OB]
                var = nrm[:, OB:2 * OB]
                scal = nrm[:, (2 + 2 * slot) * OB:(3 + 2 * slot) * OB]
                shift = nrm[:, (3 + 2 * slot) * OB:(4 + 2 * slot) * OB]
                for ob in range(OB):
                    nc.vector.reduce_sum(
                        mean[:, ob:ob + 1], ssum[:, ob * NCH:(ob + 1) * NCH],
                        axis=mybir.AxisListType.X)
                    nc.vector.reduce_sum(
                        var[:, ob:ob + 1], ssq[:, ob * NCH:(ob + 1) * NCH],
                        axis=mybir.AxisListType.X)
                nc.vector.tensor_scalar(mean, mean, 1.0 / E, None, op0=Alu.mult)
                nc.vector.tensor_scalar(var, var, 1.0 / E, None, op0=Alu.mult)
                nc.vector.scalar_tensor_tensor(
                    shift, mean, -1.0, mean, op0=Alu.mult, op1=Alu.mult)
                nc.vector.tensor_tensor(var, var, shift, op=Alu.add)
                nc.vector.tensor_scalar(var, var, EPS, None, op0=Alu.add)
                nc.scalar.activation(var, var, Act.Sqrt)
                nc.vector.reciprocal(scal, var)
                nc.vector.scalar_tensor_tensor(
                    shift, mean, -1.0, scal, op0=Alu.mult, op1=Alu.mult)
                return scal, shift

            # ------------------------- conv1 --------------------------------
            def c1_center(c):
                e0 = c * EC
                ctf = cpool.tile([128, 2, EC], dt.bfloat16, tag="ctf")
                _dma(nc.sync.dma_start(ctf[:], fd[:, :, e0:e0 + EC]))

                def get(cb):
                    if cb < 2:
                        return bufB[:, cb * E + e0:cb * E + e0 + EC]
                    return ctf[:, cb - 2, :]
                return get

            conv_pass_a(4, rm1, 2 * CO, w1, 40, 1, c1_center, bufA, stats=True)

            # c1B: x1n = relu(norm(raw1)) -> bufB ; transposes -> rm2
            scal, shift = conv_finalize(0)
            for c in range(NCH):
                e0 = c * EC
                for ob in range(OB):
                    nc.scalar.activation(
                        bufB[:, ob * E + e0:ob * E + e0 + EC],
                        bufA[:, ob * E + e0:ob * E + e0 + EC],
                        Act.Relu, bias=shift[:, ob:ob + 1],
                        scale=scal[:, ob:ob + 1])
                transpose_rows(bufB, e0, rm2, 0, OB)

            # ------------------------- conv2a -------------------------------
            def c2_center(c):
                e0 = c * EC
                return lambda cb: bufB[:, cb * E + e0:cb * E + e0 + EC]

            conv_pass_a(2, rm2, CO, w2a, 20, 2, c2_center, bufA, stats=True)

            # c2aB: x2 = relu(norm(raw2a) + x1n) -> bufB ; transposes -> rm3
            scal, shift = conv_finalize(1)
            for c in range(NCH):
                e0 = c * EC
                for ob in range(OB):
                    t = bpool.tile([128, EC], dt.bfloat16, tag="bt")
                    nc.scalar.activation(
                        t[:], bufA[:, ob * E + e0:ob * E + e0 + EC],
                        Act.Identity, bias=shift[:, ob:ob + 1],
                        scale=scal[:, ob:ob + 1])
                    nc.gpsimd.tensor_tensor(
                        t[:], t[:], bufB[:, ob * E + e0:ob * E + e0 + EC],
                        op=Alu.add)
                    nc.gpsimd.tensor_scalar(
                        bufB[:, ob * E + e0:ob * E + e0 + EC], t[:], 0.0, None,
                        op0=Alu.max)
                transpose_rows(bufB, e0, rm3, 0, OB)

            # ------------------------- conv2b -------------------------------
            conv_pass_a(2, rm3, CO, w2b, 20, 3, c2_center, bufA, stats=True)

            # c2bB: out = relu(norm(raw2b) + x2) -> DRAM f32
            scal, shift = conv_finalize(2)
            for c in range(NCH):
                e0 = c * EC
                for ob in range(OB):
                    t = bpool.tile([128, EC], dt.bfloat16, tag="bt")
                    nc.scalar.activation(
                        t[:], bufA[:, ob * E + e0:ob * E + e0 + EC],
                        Act.Identity, bias=shift[:, ob:ob + 1],
                        scale=scal[:, ob:ob + 1])
                    u = bpool.tile([128, EC], dt.float32, tag="ut")
                    nc.gpsimd.tensor_tensor(
                        u[:], t[:], bufB[:, ob * E + e0:ob * E + e0 + EC],
                        op=Alu.add)
                    nc.gpsimd.tensor_scalar(u[:], u[:], 0.0, None, op0=Alu.max)
                    _dma(nc.sync.dma_start(
                        out[ob * 128:(ob + 1) * 128, e0:e0 + EC], u[:]))

    nc.finalize()
    return nc


_NC_CACHE = {}


def _get_nc(E):
    if E not in _NC_CACHE:
        _NC_CACHE[E] = build_nc(E)
    return _NC_CACHE[E]


def make_in_maps(from_up, from_down, edge_index, W_up, b_up, W1, b1, W2a, b2a,
                 W2b, b2b, E=E_FULL):
    """Build the per-core input maps (host-side sharding + layout packing)."""
    wup_p = _pack_w(np.asarray(W_up))
    w1_p = _pack_w(np.asarray(W1))
    w2a_p = _pack_w(np.asarray(W2a))
    w2b_p = _pack_w(np.asarray(W2b))
    bia_p = np.concatenate(
        [_pack_b(b_up), _pack_b(b1), _pack_b(b2a), _pack_b(b2b)], axis=1)
    ident = np.eye(128, dtype=BF16)
    in_maps = []
    for i in range(B):
        fu_b = np.asarray(from_up[i], np.float32).astype(BF16)       # [128,E]
        fd_b = np.asarray(from_down[i], np.float32).astype(BF16)     # [256,E]
        ei = np.asarray(edge_index[i])                               # [E,4]
        nb = np.stack([fu_b[:, ei[:, s]] for s in range(4)], axis=1)  # [128,4,E]
        in_maps.append({
            "nbup": np.ascontiguousarray(nb),
            "fu": fu_b,
            "fd": np.ascontiguousarray(
                fd_b.reshape(2, 128, E).transpose(1, 0, 2)),
            "fdrm": np.ascontiguousarray(fd_b.T),
            "idx": _pack_idx(ei, E),
            "wup": wup_p, "w1": w1_p, "w2a": w2a_p, "w2b": w2b_p,
            "bia": bia_p, "ident": ident,
        })
    return in_maps


def kernel(from_up, from_down, edge_index, W_up, b_up, W1, b1, W2a, b2a,
           W2b, b2b) -> np.ndarray:
    from concourse import bass_utils

    nc = _get_nc(E_FULL)
    in_maps = make_in_maps(from_up, from_down, edge_index, W_up, b_up,
                           W1, b1, W2a, b2a, W2b, b2b)
    res = bass_utils.run_bass_kernel_spmd(nc, in_maps, core_ids=list(range(B)))
    return np.stack([r["out"] for r in res.results]).astype(np.float32)


# revision 24
# speedup vs baseline: 2.0278x; 1.0786x over previous
"""Trainium2 Bass kernel for nn_MeshUpConv (MeshCNN up-conv block).

Strategy: data-parallel over batch B=8 (one mesh per NeuronCore).

v2 changes vs baseline:
  - up-conv neighbor features are host-gathered (pure input rearrangement)
    and shipped channel-major in bf16 -> no on-device gathers for up conv,
    no fu_rm table, no input-transpose passes.
  - from_down shipped pre-cast bf16 both channel-major (centers) and
    row-major (rm1 fd-half, copied DRAM->DRAM via SBUF bounce).
  - remaining dma_gathers (conv1/conv2a/conv2b) spread across 4 SWDGE
    queues -> descriptor generation runs on 4 Q7 core pairs in parallel
    (it was the serialized bottleneck: ~9ns/row on one pair).
  - all idx chunks prefetched once (idx identical across convs).
  - feature build writes sums/absdiffs in place into the gather tiles
    (d = a-b; a' = 2a-d; |d|), no separate diff tiles.
  - plain DMAs on sync engine; gather<->plain fence kept (xbar hazard),
    per-queue gather chaining only.
"""

import sys

for _p in ("/opt/trn_rl_repo",):
    if _p not in sys.path:
        sys.path.append(_p)

import numpy as np
import ml_dtypes

BF16 = ml_dtypes.bfloat16

B = 8
E_FULL = 16384
CIN = 128
CO = 256
OB = 2          # output channel blocks of 128
EC = 512        # edges per chunk
EPS = 1e-5
GNI = 512       # idxs per dma_gather (ring-size limited)
NQ = 4          # SWDGE queues
SAFE_CHAIN = False  # chain all gathers globally (baseline hazard discipline)
PREP_TRIGGER = True  # prepare on 4 queues (parallel desc-gen), serialize fires


def _pack_idx(ei: np.ndarray, E: int) -> np.ndarray:
    """ei [E,4] int32 -> [128, NCH*128] int16 wrapped gather-index layout.

    Per chunk c the 2048 indices are ordered j = s*EC + i (slot-major), and
    index j lives at [16*g + j%16, c*128 + j//16] for every g in 0..7.
    """
    nch = E // EC
    arr = ei.reshape(nch, EC, 4).transpose(0, 2, 1).reshape(nch, 4 * EC)
    w = arr.reshape(nch, (4 * EC) // 16, 16).transpose(2, 0, 1).reshape(16, -1)
    return np.tile(w, (8, 1)).astype(np.int16)


def _pack_w(W: np.ndarray) -> np.ndarray:
    """W [256, C, 5] f32 -> [128, NBLK*128] bf16 lhsT blocks ordered (ob,k,cb)."""
    O, C, K = W.shape
    cb_n = C // 128
    out = np.empty((128, OB * K * cb_n * 128), np.float32)
    n = 0
    for ob in range(OB):
        for k in range(K):
            for cb in range(cb_n):
                blk = W[ob * 128:(ob + 1) * 128, cb * 128:(cb + 1) * 128, k].T
                out[:, n * 128:(n + 1) * 128] = blk
                n += 1
    return out.astype(BF16)


def _pack_b(b: np.ndarray) -> np.ndarray:
    return np.asarray(b).reshape(OB, 128).T.astype(np.float32).copy()


def build_nc(E: int = E_FULL):
    import concourse.bacc as bacc
    import concourse.mybir as mybir
    from concourse.tile import TileContext
    from concourse.tile_rust import add_dep_helper

    dt = mybir.dt
    Alu = mybir.AluOpType
    Act = mybir.ActivationFunctionType
    NCH = E // EC

    nc = bacc.Bacc("TRN2", num_swdge_queues=NQ)

    nbup = nc.dram_tensor("nbup", [128, 4, E], dt.bfloat16, kind="ExternalInput")
    fu = nc.dram_tensor("fu", [128, E], dt.bfloat16, kind="ExternalInput")
    fd = nc.dram_tensor("fd", [128, 2, E], dt.bfloat16, kind="ExternalInput")
    fdrm = nc.dram_tensor("fdrm", [E, CO], dt.bfloat16, kind="ExternalInput")
    idx = nc.dram_tensor("idx", [128, NCH * 128], dt.int16, kind="ExternalInput")
    wup = nc.dram_tensor("wup", [128, 10 * 128], dt.bfloat16, kind="ExternalInput")
    w1 = nc.dram_tensor("w1", [128, 40 * 128], dt.bfloat16, kind="ExternalInput")
    w2a = nc.dram_tensor("w2a", [128, 20 * 128], dt.bfloat16, kind="ExternalInput")
    w2b = nc.dram_tensor("w2b", [128, 20 * 128], dt.bfloat16, kind="ExternalInput")
    bia = nc.dram_tensor("bia", [128, 4 * OB], dt.float32, kind="ExternalInput")
    ident = nc.dram_tensor("ident", [128, 128], dt.bfloat16, kind="ExternalInput")
    out = nc.dram_tensor("out", [CO, E], dt.float32, kind="ExternalOutput")

    rm1 = nc.dram_tensor("rm1", [E, 2 * CO], dt.bfloat16, kind="Internal")
    rm2 = nc.dram_tensor("rm2", [E, CO], dt.bfloat16, kind="Internal")
    rm3 = nc.dram_tensor("rm3", [E, CO], dt.bfloat16, kind="Internal")

    # xbar-transpose gathers deadlock the SDMA engines when a plain DMA copy
    # runs concurrently. Fence: every gather group depends on all plain DMAs
    # issued since the previous group, and every later plain DMA depends on
    # the most recent gather of each queue. Same-queue gathers are ordered by
    # the per-queue ring FIFO; the 4 queues' descriptor generation runs on 4
    # Q7 core pairs in parallel.
    _pending = []
    _last_gather = [None] * NQ

    _wtail = []  # PREP_TRIGGER: last chunk's transfer-complete wait insts

    def _dma(inst):
        if PREP_TRIGGER:
            for w in _wtail:
                add_dep_helper(inst.ins, w.ins,
                               reason="dma-after-gather-fence")
        else:
            for q in range(NQ):
                if _last_gather[q] is not None:
                    add_dep_helper(inst.ins, _last_gather[q].ins,
                                   reason="dma-after-gather-fence")
        _pending.append(inst)
        return inst

    def _gather_group(insts):
        if PREP_TRIGGER:
            return  # fence deps attach to the first trigger instead
        for inst in insts:
            for d in _pending:
                add_dep_helper(inst.ins, d.ins, reason="gather-fence")
        _pending.clear()
        if SAFE_CHAIN:
            prev = _last_gather[0]
            for inst in insts:
                if prev is not None:
                    add_dep_helper(inst.ins, prev.ins, reason="gather-chain")
                prev = inst
            _last_gather[0] = prev
        else:
            for q, inst in enumerate(insts):
                _last_gather[q] = inst

    with TileContext(nc) as tc:
        with (
            tc.tile_pool(name="persist", bufs=1) as persist,
            tc.tile_pool(name="wp", bufs=1) as wpool,
            tc.tile_pool(name="gp", bufs=2) as gpool,
            tc.tile_pool(name="cp", bufs=2) as cpool,
            tc.tile_pool(name="rp", bufs=2) as rpool,
            tc.tile_pool(name="bp", bufs=2) as bpool,
            tc.tile_pool(name="jkp", bufs=1) as jkpool,
            tc.tile_pool(name="mmps", bufs=4, space="PSUM") as mmps,
            tc.tile_pool(name="tpps", bufs=2, space="PSUM") as tpps,
        ):
            bufA = persist.tile([128, OB * E], dt.bfloat16, tag="bufA")
            bufB = persist.tile([128, OB * E], dt.bfloat16, tag="bufB")
            id_t = persist.tile([128, 128], dt.bfloat16, tag="ident")
            bias_t = persist.tile([128, 4 * OB], dt.float32, tag="bias")
            ssum = persist.tile([128, OB * NCH], dt.float32, tag="ssum")
            ssq = persist.tile([128, OB * NCH], dt.float32, tag="ssq")
            nrm = persist.tile([128, 8 * OB], dt.float32, tag="nrm")
            ix_all = persist.tile([128, NCH * 128], dt.int16, tag="ix")

            _dma(nc.sync.dma_start(id_t[:], ident[:]))
            _dma(nc.sync.dma_start(bias_t[:], bia[:]))
            _dma(nc.sync.dma_start(ix_all[:], idx[:]))

            # ---------------- shared epilogue: transposes -> rm -------------
            def transpose_rows(src_buf, e0, rm_dst, col0, ncol_ob):
                """PE-transpose src_buf chunk (both ob) -> rm rows e0..e0+EC."""
                tps = []
                for ob in range(ncol_ob):
                    tp = tpps.tile([128, EC], dt.bfloat16, tag=f"tp{ob}")
                    for g in range(EC // 128):
                        nc.tensor.transpose(
                            tp[:, g * 128:(g + 1) * 128],
                            src_buf[:, ob * E + e0 + g * 128:
                                    ob * E + e0 + g * 128 + 128],
                            id_t[:])
                    tps.append(tp)
                rtc = rpool.tile([128, EC // 128, ncol_ob * 128], dt.bfloat16,
                                 tag="rtc")
                for ob in range(ncol_ob):
                    nc.vector.tensor_copy(
                        rtc[:].rearrange("p g (o n) -> p g o n", o=ncol_ob)
                        [:, :, ob, :],
                        tps[ob][:].rearrange("p (g n) -> p g n", g=EC // 128))
                for g in range(EC // 128):
                    eg = e0 + g * 128
                    _dma(nc.sync.dma_start(
                        rm_dst[eg:eg + 128, col0:col0 + ncol_ob * 128],
                        rtc[:, g, :]))

            # ------------------------- up conv ------------------------------
            wu_t = wpool.tile([128, 40 * 128], dt.bfloat16, tag="w")
            _dma(nc.sync.dma_start(wu_t[:, 0:10 * 128], wup[:]))
            for c in range(NCH):
                e0 = c * EC
                # rm1[:, 256:512] = fdrm rows (DRAM->DRAM bounce, interleaved)
                for g in range(EC // 128):
                    eg = e0 + g * 128
                    bt = rpool.tile([128, CO], dt.bfloat16, tag="fdb")
                    _dma(nc.sync.dma_start(bt[:], fdrm[eg:eg + 128, :]))
                    _dma(nc.sync.dma_start(rm1[eg:eg + 128, CO:2 * CO], bt[:]))
                nb = cpool.tile([128, 4, EC], dt.bfloat16, tag="nbu")
                _dma(nc.sync.dma_start(nb[:], nbup[:, :, e0:e0 + EC]))
                ct = cpool.tile([128, EC], dt.bfloat16, tag="ctu")
                _dma(nc.sync.dma_start(ct[:], fu[:, e0:e0 + EC]))
                # features in place: d=a-b -> slot b ; a'=2a-d -> slot a ; |d|
                nc.vector.tensor_tensor(nb[:, 2, :], nb[:, 0, :], nb[:, 2, :],
                                        op=Alu.subtract)
                nc.vector.tensor_tensor(nb[:, 3, :], nb[:, 1, :], nb[:, 3, :],
                                        op=Alu.subtract)
                nc.vector.scalar_tensor_tensor(
                    nb[:, 0, :], nb[:, 0, :], 2.0, nb[:, 2, :],
                    op0=Alu.mult, op1=Alu.subtract)
                nc.vector.scalar_tensor_tensor(
                    nb[:, 1, :], nb[:, 1, :], 2.0, nb[:, 3, :],
                    op0=Alu.mult, op1=Alu.subtract)
                for s in (2, 3):
                    di = nb[:, s, :].bitcast(dt.int16)
                    nc.vector.tensor_scalar(di, di, 0x7FFF, None,
                                            op0=Alu.bitwise_and)
                for ob in range(OB):
                    ps = mmps.tile([128, EC], dt.float32, tag="ps")
                    rhs_by_k = [ct[:], nb[:, 0, :], nb[:, 1, :],
                                nb[:, 2, :], nb[:, 3, :]]
                    for k in range(5):
                        n = ob * 5 + k
                        nc.tensor.matmul(
                            ps[:], wu_t[:, n * 128:(n + 1) * 128], rhs_by_k[k],
                            start=(k == 0), stop=(k == 4))
                    nc.scalar.activation(
                        bufB[:, ob * E + e0:ob * E + e0 + EC], ps[:],
                        Act.Identity, bias=bias_t[:, ob:ob + 1])
                transpose_rows(bufB, e0, rm1, 0, OB)

            gsems = [nc.alloc_semaphore(f"gdma{q}") for q in range(NQ)] \
                if PREP_TRIGGER else []
            gstate = {"n": 0, "tail": None, "cnt": [0] * NQ, "mm_hist": []}

            # ------------------------ conv pass A ---------------------------
            def conv_pass_a(CB, table, table_c, w_dram, nblk, bias_col,
                            center_fn, raw_dst, stats):
                w_t = wpool.tile([128, 40 * 128], dt.bfloat16, tag="w")
                _dma(nc.sync.dma_start(w_t[:, 0:nblk * 128], w_dram[:]))
                for c in range(NCH):
                    e0 = c * EC
                    gts = []
                    ginsts = []
                    for g in range(4):
                        gt = gpool.tile([128, CB, GNI], dt.bfloat16,
                                        tag=f"gd{g}")
                        ginsts.append(nc.gpsimd.dma_gather(
                            gt[:], table[:],
                            ix_all[:, c * 128 + g * (GNI // 16):
                                   c * 128 + (g + 1) * (GNI // 16)],
                            num_idxs=GNI, num_idxs_reg=GNI,
                            elem_size=table_c, transpose=True, queue_num=g,
                            prepare_only=PREP_TRIGGER,
                            sem=gsems[g] if PREP_TRIGGER else None,
                        ))
                        gts.append(gt)
                    _gather_group(ginsts)
                    dve_waits = []
                    if PREP_TRIGGER:
                        # Serialize the 4 queues' transfers: each trigger
                        # fires only after the previous gather's DMA landed
                        # (the xbar transpose path cannot run concurrently).
                        # The wait chain is transitive across queues.
                        for g in range(4):
                            prev = gstate["tail"]
                            if gstate["n"] > 0:
                                qp = (g + NQ - 1) % NQ
                                w = nc.gpsimd.wait_ge(
                                    gsems[qp], 16 * gstate["cnt"][qp])
                                if prev is not None:
                                    add_dep_helper(w.ins, prev.ins,
                                                   reason="trig-chain")
                                prev = w
                            t = nc.gpsimd.trigger_dma(count=1, queue_num=g)
                            if prev is not None:
                                add_dep_helper(t.ins, prev.ins,
                                               reason="trig-chain")
                            if g == 0:
                                # plain-DMA exclusion + table-write RAW:
                                # the first fire waits for all plains since
                                # the previous chunk's fires.
                                for d in _pending:
                                    add_dep_helper(t.ins, d.ins,
                                                   reason="gather-fence")
                                _pending.clear()
                                # WAR: this transfer overwrites the tiles the
                                # chunk-before-last's matmuls read from.
                                if len(gstate["mm_hist"]) >= 2:
                                    add_dep_helper(
                                        t.ins, gstate["mm_hist"][-2].ins,
                                        reason="gather-war")
                            gstate["tail"] = t
                            gstate["n"] += 1
                            gstate["cnt"][g] += 1
                        # RAW: consumers must wait for the transfers to land.
                        for g in range(4):
                            wv = nc.vector.wait_ge(gsems[g],
                                                   16 * gstate["cnt"][g])
                            dve_waits.append(wv)
                        _wtail.clear()
                        _wtail.extend(dve_waits)
                    # features in place (slot-major: 0,1 = n1,n2; 2,3 = n3,n4)
                    f13 = nc.vector.tensor_tensor(gts[2][:], gts[0][:],
                                                  gts[2][:], op=Alu.subtract)
                    f24 = nc.vector.tensor_tensor(gts[3][:], gts[1][:],
                                                  gts[3][:], op=Alu.subtract)
                    for wv in dve_waits:
                        add_dep_helper(f13.ins, wv.ins, reason="gather-raw")
                        add_dep_helper(f24.ins, wv.ins, reason="gather-raw")
                    nc.vector.scalar_tensor_tensor(
                        gts[0][:], gts[0][:], 2.0, gts[2][:],
                        op0=Alu.mult, op1=Alu.subtract)
                    nc.vector.scalar_tensor_tensor(
                        gts[1][:], gts[1][:], 2.0, gts[3][:],
                        op0=Alu.mult, op1=Alu.subtract)
                    for s in (2, 3):
                        di = gts[s][:].bitcast(dt.int16)
                        nc.vector.tensor_scalar(di, di, 0x7FFF, None,
                                                op0=Alu.bitwise_and)
                    ct = center_fn(c)
                    last_mm = None
                    for ob in range(OB):
                        ps = mmps.tile([128, EC], dt.float32, tag="ps")
                        nmm = 5 * CB
                        i_mm = 0
                        for k in range(5):
                            for cb in range(CB):
                                if k == 0:
                                    rhs = ct(cb)
                                else:
                                    rhs = gts[k - 1][:, cb, :]
                                n = (ob * 5 + k) * CB + cb
                                last_mm = nc.tensor.matmul(
                                    ps[:], w_t[:, n * 128:(n + 1) * 128], rhs,
                                    start=(i_mm == 0), stop=(i_mm == nmm - 1),
                                )
                                i_mm += 1
                        bias_ap = bias_t[:, bias_col * OB + ob:
                                         bias_col * OB + ob + 1]
                        raw_ap = raw_dst[:, ob * E + e0:ob * E + e0 + EC]
                        if stats:
                            nc.scalar.activation(
                                raw_ap, ps[:], Act.Identity, bias=bias_ap,
                                accum_out=ssum[:, ob * NCH + c:ob * NCH + c + 1],
                            )
                            jk = jkpool.tile([128, EC], dt.bfloat16, tag="jk")
                            nc.vector.scalar_tensor_tensor(
                                jk[:], raw_ap, 1.0, raw_ap,
                                op0=Alu.mult, op1=Alu.mult,
                                accum_out=ssq[:, ob * NCH + c:ob * NCH + c + 1],
                            )
                        else:
                            nc.scalar.activation(
                                raw_ap, ps[:], Act.Identity, bias=bias_ap,
                            )
                    gstate["mm_hist"].append(last_mm)

            # -------------------- stats finalize ---------------------------
            def conv_finalize(slot):
                mean = nrm[:, 0:OB]
                var = nrm[:, OB:2 * OB]
                scal = nrm[:, (2 + 2 * slot) * OB:(3 + 2 * slot) * OB]
                shift = nrm[:, (3 + 2 * slot) * OB:(4 + 2 * slot) * OB]
                for ob in range(OB):
                    nc.vector.reduce_sum(
                        mean[:, ob:ob + 1], ssum[:, ob * NCH:(ob + 1) * NCH],
                        axis=mybir.AxisListType.X)
                    nc.vector.reduce_sum(
                        var[:, ob:ob + 1], ssq[:, ob * NCH:(ob + 1) * NCH],
                        axis=mybir.AxisListType.X)
                nc.vector.tensor_scalar(mean, mean, 1.0 / E, None, op0=Alu.mult)
                nc.vector.tensor_scalar(var, var, 1.0 / E, None, op0=Alu.mult)
                nc.vector.scalar_tensor_tensor(
                    shift, mean, -1.0, mean, op0=Alu.mult, op1=Alu.mult)
                nc.vector.tensor_tensor(var, var, shift, op=Alu.add)
                nc.vector.tensor_scalar(var, var, EPS, None, op0=Alu.add)
                nc.scalar.activation(var, var, Act.Sqrt)
                nc.vector.reciprocal(scal, var)
                nc.vector.scalar_tensor_tensor(
                    shift, mean, -1.0, scal, op0=Alu.mult, op1=Alu.mult)
                return scal, shift

            # ------------------------- conv1 --------------------------------
            def c1_center(c):
                e0 = c * EC
                ctf = cpool.tile([128, 2, EC], dt.bfloat16, tag="ctf")
                _dma(nc.sync.dma_start(ctf[:], fd[:, :, e0:e0 + EC]))

                def get(cb):
                    if cb < 2:
                        return bufB[:, cb * E + e0:cb * E + e0 + EC]
                    return ctf[:, cb - 2, :]
                return get

            conv_pass_a(4, rm1, 2 * CO, w1, 40, 1, c1_center, bufA, stats=True)

            # c1B: x1n = relu(norm(raw1)) -> bufB ; transposes -> rm2
            scal, shift = conv_finalize(0)
            for c in range(NCH):
                e0 = c * EC
                for ob in range(OB):
                    nc.scalar.activation(
                        bufB[:, ob * E + e0:ob * E + e0 + EC],
                        bufA[:, ob * E + e0:ob * E + e0 + EC],
                        Act.Relu, bias=shift[:, ob:ob + 1],
                        scale=scal[:, ob:ob + 1])
                transpose_rows(bufB, e0, rm2, 0, OB)

            # ------------------------- conv2a -------------------------------
            def c2_center(c):
                e0 = c * EC
                return lambda cb: bufB[:, cb * E + e0:cb * E + e0 + EC]

            conv_pass_a(2, rm2, CO, w2a, 20, 2, c2_center, bufA, stats=True)

            # c2aB: x2 = relu(norm(raw2a) + x1n) -> bufB ; transposes -> rm3
            scal, shift = conv_finalize(1)
            for c in range(NCH):
                e0 = c * EC
                for ob in range(OB):
                    t = bpool.tile([128, EC], dt.bfloat16, tag="bt")
                    nc.scalar.activation(
                        t[:], bufA[:, ob * E + e0:ob * E + e0 + EC],
                        Act.Identity, bias=shift[:, ob:ob + 1],
                        scale=scal[:, ob:ob + 1])
                    nc.vector.tensor_tensor(
                        t[:], t[:], bufB[:, ob * E + e0:ob * E + e0 + EC],
                        op=Alu.add)
                    nc.vector.tensor_scalar(
                        bufB[:, ob * E + e0:ob * E + e0 + EC], t[:], 0.0, None,
                        op0=Alu.max)
                transpose_rows(bufB, e0, rm3, 0, OB)

            # ------------------------- conv2b -------------------------------
            conv_pass_a(2, rm3, CO, w2b, 20, 3, c2_center, bufA, stats=True)

            # c2bB: out = relu(norm(raw2b) + x2) -> DRAM f32
            scal, shift = conv_finalize(2)
            for c in range(NCH):
                e0 = c * EC
                for ob in range(OB):
                    t = bpool.tile([128, EC], dt.bfloat16, tag="bt")
                    nc.scalar.activation(
                        t[:], bufA[:, ob * E + e0:ob * E + e0 + EC],
                        Act.Identity, bias=shift[:, ob:ob + 1],
                        scale=scal[:, ob:ob + 1])
                    u = bpool.tile([128, EC], dt.float32, tag="ut")
                    nc.vector.tensor_tensor(
                        u[:], t[:], bufB[:, ob * E + e0:ob * E + e0 + EC],
                        op=Alu.add)
                    nc.vector.tensor_scalar(u[:], u[:], 0.0, None, op0=Alu.max)
                    _dma(nc.sync.dma_start(
                        out[ob * 128:(ob + 1) * 128, e0:e0 + EC], u[:]))

    nc.finalize()
    return nc


_NC_CACHE = {}


def _get_nc(E):
    if E not in _NC_CACHE:
        _NC_CACHE[E] = build_nc(E)
    return _NC_CACHE[E]


def make_in_maps(from_up, from_down, edge_index, W_up, b_up, W1, b1, W2a, b2a,
                 W2b, b2b, E=E_FULL):
    """Build the per-core input maps (host-side sharding + layout packing)."""
    wup_p = _pack_w(np.asarray(W_up))
    w1_p = _pack_w(np.asarray(W1))
    w2a_p = _pack_w(np.asarray(W2a))
    w2b_p = _pack_w(np.asarray(W2b))
    bia_p = np.concatenate(
        [_pack_b(b_up), _pack_b(b1), _pack_b(b2a), _pack_b(b2b)], axis=1)
    ident = np.eye(128, dtype=BF16)
    in_maps = []
    for i in range(B):
        fu_b = np.asarray(from_up[i], np.float32).astype(BF16)       # [128,E]
        fd_b = np.asarray(from_down[i], np.float32).astype(BF16)     # [256,E]
        ei = np.asarray(edge_index[i])                               # [E,4]
        nb = np.stack([fu_b[:, ei[:, s]] for s in range(4)], axis=1)  # [128,4,E]
        in_maps.append({
            "nbup": np.ascontiguousarray(nb),
            "fu": fu_b,
            "fd": np.ascontiguousarray(
                fd_b.reshape(2, 128, E).transpose(1, 0, 2)),
            "fdrm": np.ascontiguousarray(fd_b.T),
            "idx": _pack_idx(ei, E),
            "wup": wup_p, "w1": w1_p, "w2a": w2a_p, "w2b": w2b_p,
            "bia": bia_p, "ident": ident,
        })
    return in_maps


def kernel(from_up, from_down, edge_index, W_up, b_up, W1, b1, W2a, b2a,
           W2b, b2b) -> np.ndarray:
    from concourse import bass_utils

    nc = _get_nc(E_FULL)
    in_maps = make_in_maps(from_up, from_down, edge_index, W_up, b_up,
                           W1, b1, W2a, b2a, W2b, b2b)
    res = bass_utils.run_bass_kernel_spmd(nc, in_maps, core_ids=list(range(B)))
    return np.stack([r["out"] for r in res.results]).astype(np.float32)


# revision 25
# speedup vs baseline: 2.2612x; 1.1151x over previous
"""Trainium2 Bass kernel for nn_MeshUpConv (MeshCNN up-conv block).

Strategy: data-parallel over batch B=8 (one mesh per NeuronCore).

v2 changes vs baseline:
  - up-conv neighbor features are host-gathered (pure input rearrangement)
    and shipped channel-major in bf16 -> no on-device gathers for up conv,
    no fu_rm table, no input-transpose passes.
  - from_down shipped pre-cast bf16 both channel-major (centers) and
    row-major (rm1 fd-half, copied DRAM->DRAM via SBUF bounce).
  - remaining dma_gathers (conv1/conv2a/conv2b) spread across 4 SWDGE
    queues -> descriptor generation runs on 4 Q7 core pairs in parallel
    (it was the serialized bottleneck: ~9ns/row on one pair).
  - all idx chunks prefetched once (idx identical across convs).
  - feature build writes sums/absdiffs in place into the gather tiles
    (d = a-b; a' = 2a-d; |d|), no separate diff tiles.
  - plain DMAs on sync engine; gather<->plain fence kept (xbar hazard),
    per-queue gather chaining only.
"""

import sys

for _p in ("/opt/trn_rl_repo",):
    if _p not in sys.path:
        sys.path.append(_p)

import numpy as np
import ml_dtypes

BF16 = ml_dtypes.bfloat16

B = 8
E_FULL = 16384
CIN = 128
CO = 256
OB = 2          # output channel blocks of 128
EC = 512        # edges per chunk
EPS = 1e-5
GNI = 512       # idxs per dma_gather (ring-size limited)
NQ = 4          # SWDGE queues
SAFE_CHAIN = False  # chain all gathers globally (baseline hazard discipline)
PREP_TRIGGER = True  # prepare on 4 queues (parallel desc-gen), serialize fires


def _pack_idx(ei: np.ndarray, E: int) -> np.ndarray:
    """ei [E,4] int32 -> [128, NCH*128] int16 wrapped gather-index layout.

    Per chunk c the 2048 indices are ordered j = s*EC + i (slot-major), and
    index j lives at [16*g + j%16, c*128 + j//16] for every g in 0..7.
    """
    nch = E // EC
    arr = ei.reshape(nch, EC, 4).transpose(0, 2, 1).reshape(nch, 4 * EC)
    w = arr.reshape(nch, (4 * EC) // 16, 16).transpose(2, 0, 1).reshape(16, -1)
    return np.tile(w, (8, 1)).astype(np.int16)


def _pack_w(W: np.ndarray) -> np.ndarray:
    """W [256, C, 5] f32 -> [128, NBLK*128] bf16 lhsT blocks ordered (ob,k,cb)."""
    O, C, K = W.shape
    cb_n = C // 128
    out = np.empty((128, OB * K * cb_n * 128), np.float32)
    n = 0
    for ob in range(OB):
        for k in range(K):
            for cb in range(cb_n):
                blk = W[ob * 128:(ob + 1) * 128, cb * 128:(cb + 1) * 128, k].T
                out[:, n * 128:(n + 1) * 128] = blk
                n += 1
    return out.astype(BF16)


def _pack_b(b: np.ndarray) -> np.ndarray:
    return np.asarray(b).reshape(OB, 128).T.astype(np.float32).copy()


def build_nc(E: int = E_FULL):
    import concourse.bacc as bacc
    import concourse.mybir as mybir
    from concourse.tile import TileContext
    from concourse.tile_rust import add_dep_helper

    dt = mybir.dt
    Alu = mybir.AluOpType
    Act = mybir.ActivationFunctionType
    NCH = E // EC

    nc = bacc.Bacc("TRN2", num_swdge_queues=NQ)

    nbup = nc.dram_tensor("nbup", [128, 4, E], dt.bfloat16, kind="ExternalInput")
    fu = nc.dram_tensor("fu", [128, E], dt.bfloat16, kind="ExternalInput")
    fd = nc.dram_tensor("fd", [128, 2, E], dt.bfloat16, kind="ExternalInput")
    fdrm = nc.dram_tensor("fdrm", [E, CO], dt.bfloat16, kind="ExternalInput")
    idx = nc.dram_tensor("idx", [128, NCH * 128], dt.int16, kind="ExternalInput")
    wup = nc.dram_tensor("wup", [128, 10 * 128], dt.bfloat16, kind="ExternalInput")
    w1 = nc.dram_tensor("w1", [128, 40 * 128], dt.bfloat16, kind="ExternalInput")
    w2a = nc.dram_tensor("w2a", [128, 20 * 128], dt.bfloat16, kind="ExternalInput")
    w2b = nc.dram_tensor("w2b", [128, 20 * 128], dt.bfloat16, kind="ExternalInput")
    bia = nc.dram_tensor("bia", [128, 4 * OB], dt.float32, kind="ExternalInput")
    ident = nc.dram_tensor("ident", [128, 128], dt.bfloat16, kind="ExternalInput")
    out = nc.dram_tensor("out", [CO, E], dt.float32, kind="ExternalOutput")

    rm1 = nc.dram_tensor("rm1", [E, 2 * CO], dt.bfloat16, kind="Internal")
    rm2 = nc.dram_tensor("rm2", [E, CO], dt.bfloat16, kind="Internal")
    rm3 = nc.dram_tensor("rm3", [E, CO], dt.bfloat16, kind="Internal")

    # xbar-transpose gathers deadlock the SDMA engines when a plain DMA copy
    # runs concurrently. Fence: every gather group depends on all plain DMAs
    # issued since the previous group, and every later plain DMA depends on
    # the most recent gather of each queue. Same-queue gathers are ordered by
    # the per-queue ring FIFO; the 4 queues' descriptor generation runs on 4
    # Q7 core pairs in parallel.
    _pending = []
    _last_gather = [None] * NQ

    _wtail = []  # PREP_TRIGGER: last chunk's transfer-complete wait insts

    def _dma(inst):
        if PREP_TRIGGER:
            for w in _wtail:
                add_dep_helper(inst.ins, w.ins,
                               reason="dma-after-gather-fence")
        else:
            for q in range(NQ):
                if _last_gather[q] is not None:
                    add_dep_helper(inst.ins, _last_gather[q].ins,
                                   reason="dma-after-gather-fence")
        _pending.append(inst)
        return inst

    def _gather_group(insts):
        if PREP_TRIGGER:
            return  # fence deps attach to the first trigger instead
        for inst in insts:
            for d in _pending:
                add_dep_helper(inst.ins, d.ins, reason="gather-fence")
        _pending.clear()
        if SAFE_CHAIN:
            prev = _last_gather[0]
            for inst in insts:
                if prev is not None:
                    add_dep_helper(inst.ins, prev.ins, reason="gather-chain")
                prev = inst
            _last_gather[0] = prev
        else:
            for q, inst in enumerate(insts):
                _last_gather[q] = inst

    with TileContext(nc) as tc:
        with (
            tc.tile_pool(name="persist", bufs=1) as persist,
            tc.tile_pool(name="wp", bufs=1) as wpool,
            tc.tile_pool(name="gp", bufs=2) as gpool,
            tc.tile_pool(name="cp", bufs=2) as cpool,
            tc.tile_pool(name="rp", bufs=2) as rpool,
            tc.tile_pool(name="bp", bufs=2) as bpool,
            tc.tile_pool(name="jkp", bufs=1) as jkpool,
            tc.tile_pool(name="mmps", bufs=4, space="PSUM") as mmps,
            tc.tile_pool(name="tpps", bufs=2, space="PSUM") as tpps,
        ):
            bufA = persist.tile([128, OB * E], dt.bfloat16, tag="bufA")
            bufB = persist.tile([128, OB * E], dt.bfloat16, tag="bufB")
            id_t = persist.tile([128, 128], dt.bfloat16, tag="ident")
            bias_t = persist.tile([128, 4 * OB], dt.float32, tag="bias")
            ssum = persist.tile([128, OB * NCH], dt.float32, tag="ssum")
            ssq = persist.tile([128, OB * NCH], dt.float32, tag="ssq")
            nrm = persist.tile([128, 8 * OB], dt.float32, tag="nrm")
            ix_all = persist.tile([128, NCH * 128], dt.int16, tag="ix")

            _dma(nc.sync.dma_start(id_t[:], ident[:]))
            _dma(nc.sync.dma_start(bias_t[:], bia[:]))
            _dma(nc.sync.dma_start(ix_all[:], idx[:]))

            # ---------------- shared epilogue: transposes -> rm -------------
            def transpose_rows(src_buf, e0, rm_dst, col0, ncol_ob):
                """PE-transpose src_buf chunk (both ob) -> rm rows e0..e0+EC."""
                tps = []
                for ob in range(ncol_ob):
                    tp = tpps.tile([128, EC], dt.bfloat16, tag=f"tp{ob}")
                    for g in range(EC // 128):
                        nc.tensor.transpose(
                            tp[:, g * 128:(g + 1) * 128],
                            src_buf[:, ob * E + e0 + g * 128:
                                    ob * E + e0 + g * 128 + 128],
                            id_t[:])
                    tps.append(tp)
                rtc = rpool.tile([128, EC // 128, ncol_ob * 128], dt.bfloat16,
                                 tag="rtc")
                for ob in range(ncol_ob):
                    nc.vector.tensor_copy(
                        rtc[:].rearrange("p g (o n) -> p g o n", o=ncol_ob)
                        [:, :, ob, :],
                        tps[ob][:].rearrange("p (g n) -> p g n", g=EC // 128))
                _dma(nc.sync.dma_start(
                    rm_dst[e0:e0 + EC, col0:col0 + ncol_ob * 128]
                    .rearrange("(g p) c -> p g c", p=128),
                    rtc[:]))

            # ------------------------- up conv ------------------------------
            wu_t = wpool.tile([128, 40 * 128], dt.bfloat16, tag="w")
            _dma(nc.sync.dma_start(wu_t[:, 0:10 * 128], wup[:]))
            for c in range(NCH):
                e0 = c * EC
                # rm1[:, 256:512] = fdrm rows (DRAM->DRAM bounce, interleaved)
                bt = rpool.tile([128, EC // 128, CO], dt.bfloat16, tag="fdb")
                _dma(nc.sync.dma_start(
                    bt[:], fdrm[e0:e0 + EC, :]
                    .rearrange("(g p) c -> p g c", p=128)))
                _dma(nc.sync.dma_start(
                    rm1[e0:e0 + EC, CO:2 * CO]
                    .rearrange("(g p) c -> p g c", p=128), bt[:]))
                nb = cpool.tile([128, 4, EC], dt.bfloat16, tag="nbu")
                _dma(nc.sync.dma_start(nb[:], nbup[:, :, e0:e0 + EC]))
                ct = cpool.tile([128, EC], dt.bfloat16, tag="ctu")
                _dma(nc.sync.dma_start(ct[:], fu[:, e0:e0 + EC]))
                # features in place: d=a-b -> slot b ; a'=2a-d -> slot a ; |d|
                nc.vector.tensor_tensor(nb[:, 2, :], nb[:, 0, :], nb[:, 2, :],
                                        op=Alu.subtract)
                nc.vector.tensor_tensor(nb[:, 3, :], nb[:, 1, :], nb[:, 3, :],
                                        op=Alu.subtract)
                nc.vector.scalar_tensor_tensor(
                    nb[:, 0, :], nb[:, 0, :], 2.0, nb[:, 2, :],
                    op0=Alu.mult, op1=Alu.subtract)
                nc.vector.scalar_tensor_tensor(
                    nb[:, 1, :], nb[:, 1, :], 2.0, nb[:, 3, :],
                    op0=Alu.mult, op1=Alu.subtract)
                for s in (2, 3):
                    di = nb[:, s, :].bitcast(dt.int16)
                    nc.vector.tensor_scalar(di, di, 0x7FFF, None,
                                            op0=Alu.bitwise_and)
                for ob in range(OB):
                    ps = mmps.tile([128, EC], dt.float32, tag="ps")
                    rhs_by_k = [ct[:], nb[:, 0, :], nb[:, 1, :],
                                nb[:, 2, :], nb[:, 3, :]]
                    for k in range(5):
                        n = ob * 5 + k
                        nc.tensor.matmul(
                            ps[:], wu_t[:, n * 128:(n + 1) * 128], rhs_by_k[k],
                            start=(k == 0), stop=(k == 4))
                    nc.scalar.activation(
                        bufB[:, ob * E + e0:ob * E + e0 + EC], ps[:],
                        Act.Identity, bias=bias_t[:, ob:ob + 1])
                transpose_rows(bufB, e0, rm1, 0, OB)

            gsems = [nc.alloc_semaphore(f"gdma{q}") for q in range(NQ)] \
                if PREP_TRIGGER else []
            gstate = {"n": 0, "tail": None, "cnt": [0] * NQ, "mm_hist": []}

            # ------------------------ conv pass A ---------------------------
            def conv_pass_a(CB, table, table_c, w_dram, nblk, bias_col,
                            center_fn, raw_dst, stats):
                w_t = wpool.tile([128, 40 * 128], dt.bfloat16, tag="w")
                _dma(nc.sync.dma_start(w_t[:, 0:nblk * 128], w_dram[:]))
                for c in range(NCH):
                    e0 = c * EC
                    gts = []
                    ginsts = []
                    for g in range(4):
                        gt = gpool.tile([128, CB, GNI], dt.bfloat16,
                                        tag=f"gd{g}")
                        ginsts.append(nc.gpsimd.dma_gather(
                            gt[:], table[:],
                            ix_all[:, c * 128 + g * (GNI // 16):
                                   c * 128 + (g + 1) * (GNI // 16)],
                            num_idxs=GNI, num_idxs_reg=GNI,
                            elem_size=table_c, transpose=True, queue_num=g,
                            prepare_only=PREP_TRIGGER,
                            sem=gsems[g] if PREP_TRIGGER else None,
                        ))
                        gts.append(gt)
                    _gather_group(ginsts)
                    dve_waits = []
                    if PREP_TRIGGER:
                        # Serialize the 4 queues' transfers: each trigger
                        # fires only after the previous gather's DMA landed
                        # (the xbar transpose path cannot run concurrently).
                        # The wait chain is transitive across queues.
                        for g in range(4):
                            prev = gstate["tail"]
                            if gstate["n"] > 0:
                                qp = (g + NQ - 1) % NQ
                                w = nc.gpsimd.wait_ge(
                                    gsems[qp], 16 * gstate["cnt"][qp])
                                if prev is not None:
                                    add_dep_helper(w.ins, prev.ins,
                                                   reason="trig-chain")
                                prev = w
                            t = nc.gpsimd.trigger_dma(count=1, queue_num=g)
                            if prev is not None:
                                add_dep_helper(t.ins, prev.ins,
                                               reason="trig-chain")
                            if g == 0:
                                # plain-DMA exclusion + table-write RAW:
                                # the first fire waits for all plains since
                                # the previous chunk's fires.
                                for d in _pending:
                                    add_dep_helper(t.ins, d.ins,
                                                   reason="gather-fence")
                                _pending.clear()
                                # WAR: this transfer overwrites the tiles the
                                # chunk-before-last's matmuls read from.
                                if len(gstate["mm_hist"]) >= 2:
                                    add_dep_helper(
                                        t.ins, gstate["mm_hist"][-2].ins,
                                        reason="gather-war")
                            gstate["tail"] = t
                            gstate["n"] += 1
                            gstate["cnt"][g] += 1
                        # RAW: consumers must wait for the transfers to land.
                        for g in range(4):
                            wv = nc.vector.wait_ge(gsems[g],
                                                   16 * gstate["cnt"][g])
                            dve_waits.append(wv)
                        _wtail.clear()
                        _wtail.extend(dve_waits)
                    # features in place (slot-major: 0,1 = n1,n2; 2,3 = n3,n4)
                    f13 = nc.vector.tensor_tensor(gts[2][:], gts[0][:],
                                                  gts[2][:], op=Alu.subtract)
                    f24 = nc.vector.tensor_tensor(gts[3][:], gts[1][:],
                                                  gts[3][:], op=Alu.subtract)
                    for wv in dve_waits:
                        add_dep_helper(f13.ins, wv.ins, reason="gather-raw")
                        add_dep_helper(f24.ins, wv.ins, reason="gather-raw")
                    nc.vector.scalar_tensor_tensor(
                        gts[0][:], gts[0][:], 2.0, gts[2][:],
                        op0=Alu.mult, op1=Alu.subtract)
                    nc.vector.scalar_tensor_tensor(
                        gts[1][:], gts[1][:], 2.0, gts[3][:],
                        op0=Alu.mult, op1=Alu.subtract)
                    for s in (2, 3):
                        di = gts[s][:].bitcast(dt.int16)
                        nc.vector.tensor_scalar(di, di, 0x7FFF, None,
                                                op0=Alu.bitwise_and)
                    ct = center_fn(c)
                    last_mm = None
                    for ob in range(OB):
                        ps = mmps.tile([128, EC], dt.float32, tag="ps")
                        nmm = 5 * CB
                        i_mm = 0
                        for k in range(5):
                            for cb in range(CB):
                                if k == 0:
                                    rhs = ct(cb)
                                else:
                                    rhs = gts[k - 1][:, cb, :]
                                n = (ob * 5 + k) * CB + cb
                                last_mm = nc.tensor.matmul(
                                    ps[:], w_t[:, n * 128:(n + 1) * 128], rhs,
                                    start=(i_mm == 0), stop=(i_mm == nmm - 1),
                                )
                                i_mm += 1
                        bias_ap = bias_t[:, bias_col * OB + ob:
                                         bias_col * OB + ob + 1]
                        raw_ap = raw_dst[:, ob * E + e0:ob * E + e0 + EC]
                        if stats:
                            nc.scalar.activation(
                                raw_ap, ps[:], Act.Identity, bias=bias_ap,
                                accum_out=ssum[:, ob * NCH + c:ob * NCH + c + 1],
                            )
                            jk = jkpool.tile([128, EC], dt.bfloat16, tag="jk")
                            nc.vector.scalar_tensor_tensor(
                                jk[:], raw_ap, 1.0, raw_ap,
                                op0=Alu.mult, op1=Alu.mult,
                                accum_out=ssq[:, ob * NCH + c:ob * NCH + c + 1],
                            )
                        else:
                            nc.scalar.activation(
                                raw_ap, ps[:], Act.Identity, bias=bias_ap,
                            )
                    gstate["mm_hist"].append(last_mm)

            # -------------------- stats finalize ---------------------------
            def conv_finalize(slot):
                mean = nrm[:, 0:OB]
                var = nrm[:, OB:2 * OB]
                scal = nrm[:, (2 + 2 * slot) * OB:(3 + 2 * slot) * OB]
                shift = nrm[:, (3 + 2 * slot) * OB:(4 + 2 * slot) * OB]
                for ob in range(OB):
                    nc.vector.reduce_sum(
                        mean[:, ob:ob + 1], ssum[:, ob * NCH:(ob + 1) * NCH],
                        axis=mybir.AxisListType.X)
                    nc.vector.reduce_sum(
                        var[:, ob:ob + 1], ssq[:, ob * NCH:(ob + 1) * NCH],
                        axis=mybir.AxisListType.X)
                nc.vector.tensor_scalar(mean, mean, 1.0 / E, None, op0=Alu.mult)
                nc.vector.tensor_scalar(var, var, 1.0 / E, None, op0=Alu.mult)
                nc.vector.scalar_tensor_tensor(
                    shift, mean, -1.0, mean, op0=Alu.mult, op1=Alu.mult)
                nc.vector.tensor_tensor(var, var, shift, op=Alu.add)
                nc.vector.tensor_scalar(var, var, EPS, None, op0=Alu.add)
                nc.scalar.activation(var, var, Act.Sqrt)
                nc.vector.reciprocal(scal, var)
                nc.vector.scalar_tensor_tensor(
                    shift, mean, -1.0, scal, op0=Alu.mult, op1=Alu.mult)
                return scal, shift

            # ------------------------- conv1 --------------------------------
            def c1_center(c):
                e0 = c * EC
                ctf = cpool.tile([128, 2, EC], dt.bfloat16, tag="ctf")
                _dma(nc.sync.dma_start(ctf[:], fd[:, :, e0:e0 + EC]))

                def get(cb):
                    if cb < 2:
                        return bufB[:, cb * E + e0:cb * E + e0 + EC]
                    return ctf[:, cb - 2, :]
                return get

            conv_pass_a(4, rm1, 2 * CO, w1, 40, 1, c1_center, bufA, stats=True)

            # c1B: x1n = relu(norm(raw1)) -> bufB ; transposes -> rm2
            scal, shift = conv_finalize(0)
            for c in range(NCH):
                e0 = c * EC
                for ob in range(OB):
                    nc.scalar.activation(
                        bufB[:, ob * E + e0:ob * E + e0 + EC],
                        bufA[:, ob * E + e0:ob * E + e0 + EC],
                        Act.Relu, bias=shift[:, ob:ob + 1],
                        scale=scal[:, ob:ob + 1])
                transpose_rows(bufB, e0, rm2, 0, OB)

            # ------------------------- conv2a -------------------------------
            def c2_center(c):
                e0 = c * EC
                return lambda cb: bufB[:, cb * E + e0:cb * E + e0 + EC]

            conv_pass_a(2, rm2, CO, w2a, 20, 2, c2_center, bufA, stats=True)

            # c2aB: x2 = relu(norm(raw2a) + x1n) -> bufB ; transposes -> rm3
            scal, shift = conv_finalize(1)
            for c in range(NCH):
                e0 = c * EC
                for ob in range(OB):
                    t = bpool.tile([128, EC], dt.bfloat16, tag="bt")
                    nc.scalar.activation(
                        t[:], bufA[:, ob * E + e0:ob * E + e0 + EC],
                        Act.Identity, bias=shift[:, ob:ob + 1],
                        scale=scal[:, ob:ob + 1])
                    nc.vector.tensor_tensor(
                        t[:], t[:], bufB[:, ob * E + e0:ob * E + e0 + EC],
                        op=Alu.add)
                    nc.vector.tensor_scalar(
                        bufB[:, ob * E + e0:ob * E + e0 + EC], t[:], 0.0, None,
                        op0=Alu.max)
                transpose_rows(bufB, e0, rm3, 0, OB)

            # ------------------------- conv2b -------------------------------
            conv_pass_a(2, rm3, CO, w2b, 20, 3, c2_center, bufA, stats=True)

            # c2bB: out = relu(norm(raw2b) + x2) -> DRAM f32
            scal, shift = conv_finalize(2)
            for c in range(NCH):
                e0 = c * EC
                for ob in range(OB):
                    t = bpool.tile([128, EC], dt.bfloat16, tag="bt")
                    nc.scalar.activation(
                        t[:], bufA[:, ob * E + e0:ob * E + e0 + EC],
                        Act.Identity, bias=shift[:, ob:ob + 1],
                        scale=scal[:, ob:ob + 1])
                    u = bpool.tile([128, EC], dt.float32, tag="ut")
                    nc.vector.tensor_tensor(
                        u[:], t[:], bufB[:, ob * E + e0:ob * E + e0 + EC],
                        op=Alu.add)
                    nc.vector.tensor_scalar(u[:], u[:], 0.0, None, op0=Alu.max)
                    _dma(nc.sync.dma_start(
                        out[ob * 128:(ob + 1) * 128, e0:e0 + EC], u[:]))

    nc.finalize()
    return nc


_NC_CACHE = {}


def _get_nc(E):
    if E not in _NC_CACHE:
        _NC_CACHE[E] = build_nc(E)
    return _NC_CACHE[E]


def make_in_maps(from_up, from_down, edge_index, W_up, b_up, W1, b1, W2a, b2a,
                 W2b, b2b, E=E_FULL):
    """Build the per-core input maps (host-side sharding + layout packing)."""
    wup_p = _pack_w(np.asarray(W_up))
    w1_p = _pack_w(np.asarray(W1))
    w2a_p = _pack_w(np.asarray(W2a))
    w2b_p = _pack_w(np.asarray(W2b))
    bia_p = np.concatenate(
        [_pack_b(b_up), _pack_b(b1), _pack_b(b2a), _pack_b(b2b)], axis=1)
    ident = np.eye(128, dtype=BF16)
    in_maps = []
    for i in range(B):
        fu_b = np.asarray(from_up[i], np.float32).astype(BF16)       # [128,E]
        fd_b = np.asarray(from_down[i], np.float32).astype(BF16)     # [256,E]
        ei = np.asarray(edge_index[i])                               # [E,4]
        nb = np.stack([fu_b[:, ei[:, s]] for s in range(4)], axis=1)  # [128,4,E]
        in_maps.append({
            "nbup": np.ascontiguousarray(nb),
            "fu": fu_b,
            "fd": np.ascontiguousarray(
                fd_b.reshape(2, 128, E).transpose(1, 0, 2)),
            "fdrm": np.ascontiguousarray(fd_b.T),
            "idx": _pack_idx(ei, E),
            "wup": wup_p, "w1": w1_p, "w2a": w2a_p, "w2b": w2b_p,
            "bia": bia_p, "ident": ident,
        })
    return in_maps


def kernel(from_up, from_down, edge_index, W_up, b_up, W1, b1, W2a, b2a,
           W2b, b2b) -> np.ndarray:
    from concourse import bass_utils

    nc = _get_nc(E_FULL)
    in_maps = make_in_maps(from_up, from_down, edge_index, W_up, b_up,
                           W1, b1, W2a, b2a, W2b, b2b)
    res = bass_utils.run_bass_kernel_spmd(nc, in_maps, core_ids=list(range(B)))
    return np.stack([r["out"] for r in res.results]).astype(np.float32)
